# revision 1
# baseline (speedup 1.0000x reference)
"""Causal self-attention on 8 NeuronCores (Bass/Tile).

Sharding: tensor-parallel over heads x data-parallel over batch.
  core c -> batch b = c//4, heads 4g..4g+3 where g = c%4.
Each core computes q,k,v for its 4 heads (over its batch's 2048 tokens),
causal softmax attention, and the partial output projection over its 256
head-channels. Host sums the 4 partials per batch and adds b_proj.

v2 design (cost model: matmul time = out_free_size x cyc/row, rhs dtype
keyed; bf16 = 1 cyc/row at any width):
- QKV projection in f32r (full rate at free>=256), biases folded in via
  DVE adds that double as the psum->sbuf drain; q/k land in bf16.
- Scores S[kt,q] in bf16 (64-deep contraction, psum [128, 2 heads, 512]),
  exp fused across 2 heads per ACT instruction (halves ACT instr count).
- p@v computed TRANSPOSED: py[q, d+1] = p[kt,q]^T @ vaug[kt, d|1], so the
  matmul free dim is 65 instead of the 128..512 query width -- pv PE rows
  drop 2x. Denominator rides along as vaug's ones column; the normalize
  becomes a per-partition-scalar broadcast multiply (no partition
  broadcast needed). y is then transposed back with PE transpose-mode
  matmuls (128 rows each) for the output projection.
- Output projection in bf16 from the transposed yT, drained by DVE,
  written f32 to DRAM.
The per-512-token stripes are emitted interleaved; proj for q-tile tg is
emitted one tile behind (lag-1) so its PE work fills the ACT-bound tail.
"""

import os
import sys

for _p in ("/opt/trn_rl_repo", "/opt/pypackages"):
    if os.path.isdir(_p) and _p not in sys.path:
        sys.path.append(_p)

import numpy as np

import concourse.bass as bass
import concourse.tile as tile
import concourse.mybir as mybir
from concourse import bacc
from concourse.bass_utils import run_bass_kernel_spmd

B, T, C = 2, 2048, 1024
H = 16            # total heads
D = 64            # head dim
HPC = 4           # heads per core
CH = HPC * D      # 256 channels per core
N_CORES = 8

f32 = mybir.dt.float32
f32r = mybir.dt.float32r
bf16 = mybir.dt.bfloat16
ts = bass.ts
ds = bass.ds
AF = mybir.ActivationFunctionType

_COMPILED = None

# emission-policy knobs (sweepable via env for TimelineSim experiments)
POLICY = {
    # 0: four single chains; 1: two interleaved pairs; 2: pairs stripe0 only
    "qk_pair": int(os.environ.get("K_QK_PAIR", "0")),
    "norm_lag": int(os.environ.get("K_NORM_LAG", "1")),
    "yt_in_big": int(os.environ.get("K_YT_IN_BIG", "0")),
    "spend_proj": os.environ.get("K_SPEND_PROJ", "s3"),  # none|s3|all
    "dma_t": int(os.environ.get("K_DMA_T", "0")),  # yT transpose via DMA xbar
    "qk_bias_act": int(os.environ.get("K_QK_BIAS_ACT", "0")),
    "warmup": int(os.environ.get("K_WARMUP", "0")),  # dummy PE ramp matmuls
    "tail_split": int(os.environ.get("K_TAIL_SPLIT", "0")),
    "double_fill": int(os.environ.get("K_DOUBLE_FILL", "0")),  # s3 2 fillers
    "sc_bufs": int(os.environ.get("K_SC_BUFS", "2")),
    "big_bufs": int(os.environ.get("K_BIG_BUFS", "2")),
    "masks_pool": int(os.environ.get("K_MASKS_POOL", "0")),
    "v3_defer": int(os.environ.get("K_V3_DEFER", "1")),
    "out_bf16": int(os.environ.get("K_OUT_BF16", "1")),
    "adv_fill": int(os.environ.get("K_ADV_FILL", "1")),
    "fast_start": int(os.environ.get("K_FAST_START", "0")),
    "pv_split": int(os.environ.get("K_PV_SPLIT", "0")),
    "exp4": int(os.environ.get("K_EXP4", "0")),  # 4-head exp, 1 sc buf
    "proj_fine": int(os.environ.get("K_PROJ_FINE", "0")),  # 256-wide po units
    "py_bufs": int(os.environ.get("K_PY_BUFS", "2")),
    "xt_prefetch": int(os.environ.get("K_XT_PREFETCH", "1")),  # stripes ahead
    # 1: proj drain oi=1 on ACT everywhere; 2: only in the end flush
    "drain_alt": int(os.environ.get("K_DRAIN_ALT", "2")),
    "tail_fine": int(os.environ.get("K_TAIL_FINE", "1")),  # slab-split tile 15
    "slot_swap": int(os.environ.get("K_SLOT_SWAP", "0")),  # filler before scores
    "proj_merge": int(os.environ.get("K_PROJ_MERGE", "1")),  # 1 DMA per proj tg
    "v_defer_all": int(os.environ.get("K_V_DEFER_ALL", "1")),  # v as filler ti>=1
    "s0_gsplit": int(os.environ.get("K_S0_GSPLIT", "0")),  # early g0 scores s0
    "qkv_bf16": int(os.environ.get("K_QKV_BF16", "1")),  # x/W_attn in bf16
    "xw_merge": int(os.environ.get("K_XW_MERGE", "1")),  # fused x0|wqk loads
}


def _build():
    nc = bacc.Bacc("TRN2", target_bir_lowering=False, debug=False,
                   num_devices=N_CORES)

    xw_dt = bf16 if POLICY["qkv_bf16"] else f32
    xT = nc.dram_tensor("xT", [C, T], xw_dt, kind="ExternalInput").ap()
    wt = nc.dram_tensor("wt", [C, 3 * CH], xw_dt, kind="ExternalInput").ap()
    if POLICY["xw_merge"]:
        # stripe-0 x and the q/k weight half concatenated per channel row:
        # one DMA per ci chunk instead of two (startup is HWDGE-bound)
        xw0 = nc.dram_tensor("xw0", [C, 1024], bf16,
                             kind="ExternalInput").ap()
    if POLICY["fast_start"]:
        # bf16 copies of stripe-0 x and the q/k weight half: 4x fewer
        # startup bytes, so the first score matmuls start ~5us sooner.
        x0b = nc.dram_tensor("x0b", [C, 512], bf16, kind="ExternalInput").ap()
        wqkb = nc.dram_tensor("wqkb", [C, 512], bf16,
                              kind="ExternalInput").ap()
    wpt = nc.dram_tensor("wpt", [CH, C], f32, kind="ExternalInput").ap()
    bqk = nc.dram_tensor("bqk", [128, 4], f32, kind="ExternalInput").ap()
    bvb = nc.dram_tensor("bvb", [128, HPC, D], f32, kind="ExternalInput").ap()
    Sm = nc.dram_tensor("Sm", [128, 128], f32, kind="ExternalInput").ap()
    Idm = nc.dram_tensor("Idm", [128, 128], f32, kind="ExternalInput").ap()
    out_dt = bf16 if POLICY["out_bf16"] else f32
    out = nc.dram_tensor("out_partial", [T, C], out_dt,
                         kind="ExternalOutput").ap()

    NT512 = T // 512          # 4   512-token stripes
    NT128 = T // 128          # 16  128-token tiles
    NC128 = C // 128          # 8   contraction tiles

    with tile.TileContext(nc) as tc:
        with tc.tile_pool(name="consts", bufs=1) as consts, \
             tc.tile_pool(name="qkv", bufs=1) as qkv, \
             tc.tile_pool(name="xp",
                          bufs=POLICY["xt_prefetch"] + 1) as xp, \
             tc.tile_pool(name="pp", bufs=17) as pp, \
             tc.tile_pool(name="yn", bufs=2) as yn, \
             tc.tile_pool(name="op", bufs=3) as op, \
             tc.tile_pool(name="ps_s", bufs=POLICY["sc_bufs"],
                          space="PSUM") as ps_s, \
             tc.tile_pool(name="ps_y", bufs=POLICY["py_bufs"],
                          space="PSUM") as ps_y, \
             tc.tile_pool(name="ps_big", bufs=POLICY["big_bufs"],
                          space="PSUM") as ps_big:

            # ---- constants; DMA emission order puts stripe-0 essentials
            #      (xt0 chunks + qk half of wt) first ----
            mm_dt = bf16 if POLICY["qkv_bf16"] else f32r
            xT_r = xT.rearrange("(o p) t -> p o t", p=128)
            wt_r = wt.rearrange("(o p) f -> p o f", p=128)
            if not POLICY["qkv_bf16"]:
                xT_r = xT_r.bitcast(f32r)
                wt_r = wt_r.bitcast(f32r)
            wt_sb = consts.tile([128, NC128, 3 * CH], mm_dt)
            xt0 = xp.tile([128, NC128, 512], mm_dt, tag="xt")
            x0b_sb = wqkb_sb = None
            xw_sb = None
            if POLICY["xw_merge"]:
                xw_r = xw0.rearrange("(o p) f -> p o f", p=128)
                xw_sb = consts.tile([128, NC128, 1024], bf16)
                for ci in range(NC128):
                    nc.sync.dma_start(xw_sb[:, ci], xw_r[:, ci])
                bqk_sb = consts.tile([128, 4], f32)
                nc.sync.dma_start(bqk_sb[:], bqk)
                nc.sync.dma_start(wt_sb[:, :, 512:], wt_r[:, :, 512:])
            elif POLICY["fast_start"]:
                x0b_r = x0b.rearrange("(o p) t -> p o t", p=128)
                wqkb_r = wqkb.rearrange("(o p) f -> p o f", p=128)
                x0b_sb = consts.tile([128, NC128, 512], bf16)
                wqkb_sb = consts.tile([128, NC128, 512], bf16)
                for hf in range(2):
                    nc.sync.dma_start(
                        x0b_sb[:, ts(hf, 4)], x0b_r[:, ts(hf, 4)])
                    nc.sync.dma_start(
                        wqkb_sb[:, ts(hf, 4)], wqkb_r[:, ts(hf, 4)])
                bqk_sb = consts.tile([128, 4], f32)
                nc.sync.dma_start(bqk_sb[:], bqk)
                nc.sync.dma_start(wt_sb[:, :, 512:], wt_r[:, :, 512:])
                nc.sync.dma_start(xt0[:], xT_r[:, :, ts(0, 512)])
                nc.sync.dma_start(wt_sb[:, :, :512], wt_r[:, :, :512])
            else:
                for ci in range(NC128):
                    nc.sync.dma_start(xt0[:, ci], xT_r[:, ci, ts(0, 512)])
                    nc.sync.dma_start(wt_sb[:, ci, :512], wt_r[:, ci, :512])
                bqk_sb = consts.tile([128, 4], f32)
                nc.sync.dma_start(bqk_sb[:], bqk)
                nc.sync.dma_start(wt_sb[:, :, 512:], wt_r[:, :, 512:])
            bvb_sb = consts.tile([128, HPC, D], f32)
            nc.sync.dma_start(bvb_sb[:], bvb)
            S_f = consts.tile([128, 128], f32)
            nc.sync.dma_start(S_f[:], Sm)
            Id_f = consts.tile([128, 128], f32)
            nc.sync.dma_start(Id_f[:], Idm)
            wpt_f = consts.tile([128, 2, C], f32)
            nc.sync.dma_start(
                wpt_f[:], wpt.rearrange("(s p) o -> p s o", p=128))

            S_sb = consts.tile([128, 128], bf16)
            nc.vector.tensor_copy(S_sb[:], S_f[:])
            Id_sb = consts.tile([128, 128], bf16)
            nc.vector.tensor_copy(Id_sb[:], Id_f[:])
            wpt_sb = consts.tile([128, 2, C], bf16)
            nc.vector.tensor_copy(wpt_sb[:], wpt_f[:])

            # ---- persistent activations ----
            qT = qkv.tile([128, 2, T], bf16)      # [2h*64, slab, t]
            kT = qkv.tile([128, 2, T], bf16)
            vaug = qkv.tile([128, NT128, HPC, D + 1], bf16)  # [kt, ki, h, d|1]
            yT = qkv.tile([128, 2, T], bf16)

            nc.vector.memset(vaug[:, :, :, D:D + 1], 1.0)

            if POLICY["warmup"]:
                # dummy matmuls on a zeroed tile: occupy the (DMA-starved)
                # PE during startup so the p-state ramp completes before the
                # real chains arrive.
                warm = consts.tile([128, 512], bf16)
                nc.vector.memset(warm[:], 0.0)
                wps = ps_big.tile([128, 512], f32, tag="big")
                for _ in range(POLICY["warmup"]):
                    nc.tensor.matmul(wps[:], warm[:, :128], warm[:],
                                     start=True, stop=True)

            # ---------------- emission helpers ----------------
            from collections import deque

            xt_tiles = ({0: xw_sb[:, :, 0:512]} if POLICY["xw_merge"]
                        else {0: xt0})

            def ensure_xt_dma(ti):
                if ti < NT512 and ti not in xt_tiles:
                    xt = xp.tile([128, NC128, 512], mm_dt, tag="xt")
                    nc.sync.dma_start(xt[:], xT_r[:, :, ts(ti, 512)])
                    xt_tiles[ti] = xt

            def _qk_srcs(ti):
                if POLICY["xw_merge"]:
                    xt = (xw_sb[:, :, 0:512] if ti == 0 else xt_tiles[ti])
                    return xt, xw_sb[:, :, 512:1024]
                if ti == 0 and POLICY["fast_start"]:
                    return x0b_sb, wqkb_sb
                return xt_tiles[ti], wt_sb

            def emit_qk_pair(ti, slab):
                # q and k chains for one slab, interleaved by ci so a
                # DMA-paced stripe keeps both accumulations advancing.
                xt, wq = _qk_srcs(ti)
                psq = ps_big.tile([128, 512], f32, tag="big")
                psk = ps_big.tile([128, 512], f32, tag="big")
                for ci in range(NC128):
                    nc.tensor.matmul(
                        psq[:], wq[:, ci, ts(slab, 128)], xt[:, ci, :],
                        start=(ci == 0), stop=(ci == NC128 - 1))
                    nc.tensor.matmul(
                        psk[:], wq[:, ci, ts(2 + slab, 128)], xt[:, ci, :],
                        start=(ci == 0), stop=(ci == NC128 - 1))
                _qk_drain(psq, qT, slab, slab, ti)
                _qk_drain(psk, kT, 2 + slab, slab, ti)

            def _qk_drain(ps, dest, fj, slab, ti):
                # psum -> bf16 sbuf with per-channel bias. ACT variant frees
                # DVE (runs in the QKV windows where ACT is otherwise idle).
                if POLICY["qk_bias_act"]:
                    nc.scalar.activation(
                        dest[:, slab, ts(ti, 512)], ps[:], AF.Identity,
                        bias=bqk_sb[:, fj:fj + 1])
                else:
                    nc.vector.tensor_add(
                        out=dest[:, slab, ts(ti, 512)], in0=ps[:],
                        in1=bqk_sb[:, fj:fj + 1].to_broadcast([128, 512]))

            def emit_qk_chain(ti, fj):
                xt, wq = _qk_srcs(ti)
                ps = ps_big.tile([128, 512], f32, tag="big")
                for ci in range(NC128):
                    nc.tensor.matmul(
                        ps[:], wq[:, ci, ts(fj, 128)], xt[:, ci, :],
                        start=(ci == 0), stop=(ci == NC128 - 1))
                _qk_drain(ps, qT if fj < 2 else kT, fj, fj % 2, ti)

            def qk_units(ti):
                qp = POLICY["qk_pair"]
                if qp == 1 or (qp == 2 and ti == 0):
                    return [(emit_qk_pair, (ti, 0)), (emit_qk_pair, (ti, 1))]
                return [(emit_qk_chain, (ti, fj)) for fj in range(4)]

            def emit_qk_units(ti):
                if ti == 0 and POLICY["s0_gsplit"]:
                    # q0,k0 first, then slab-0 scores for all of stripe 0 so
                    # ACT starts ~3us earlier; q1,k1 follow, slab-1 scores
                    # come from the normal loop.
                    emit_qk_chain(0, 0)
                    emit_qk_chain(0, 2)
                    for ki in range(4):
                        emit_scores(0, ki, groups=(0,))
                    emit_qk_chain(0, 1)
                    emit_qk_chain(0, 3)
                    return
                for fn, args in qk_units(ti):
                    fn(*args)

            def emit_v_chain(ti, tj):
                xt = xt_tiles[ti]
                pv = ps_big.tile([128, HPC, D], f32, tag="big")
                for ci in range(NC128):
                    nc.tensor.matmul(
                        pv[:, :, :], xt[:, ci, ts(tj, 128)],
                        wt_sb[:, ci, 512:512 + CH],
                        start=(ci == 0), stop=(ci == NC128 - 1))
                nc.vector.tensor_add(
                    out=vaug[:, 4 * ti + tj, :, 0:D],
                    in0=pv[:, :, :], in1=bvb_sb[:])

            p4_all = {}  # (stripe, ki) -> p4 tile

            def emit_scores(qi, ki, groups=(0, 1)):
                j = ki - 4 * qi
                q0 = max(0, 128 * j)
                w = 512 - q0
                if (qi, ki) in p4_all:
                    p4 = p4_all[(qi, ki)]
                else:
                    p4 = pp.tile([128, HPC, 512], bf16, tag="p4")
                if POLICY["exp4"]:
                    # one 4-bank psum tile + one fused exp for all 4 heads:
                    # halves the ACT instruction count for the same psum
                    # footprint (bufs=1 x 4 banks vs bufs=2 x 2 banks).
                    sc = ps_s.tile([128, HPC, 512], f32, tag="sc", bufs=1)
                    for g in range(2):
                        for hh in range(2):
                            nc.tensor.matmul(
                                sc[:, 2 * g + hh, q0:],
                                kT[ts(hh, D), g, ts(ki, 128)],
                                qT[ts(hh, D), g, ds(512 * qi + q0, w)],
                                start=True, stop=True)
                    nc.scalar.activation(
                        p4[:, :, q0:], sc[:, :, q0:], AF.Exp)
                    if j >= 0:
                        eng = nc.gpsimd if POLICY["masks_pool"] else nc.vector
                        for h in range(HPC):
                            eng.tensor_mul(
                                out=p4[:, h, q0:q0 + 128],
                                in0=p4[:, h, q0:q0 + 128],
                                in1=S_sb[:])
                    p4_all[(qi, ki)] = p4
                    return
                for g in groups:
                    sc = ps_s.tile([128, 2, 512], f32, tag="sc")
                    for hh in range(2):
                        nc.tensor.matmul(
                            sc[:, hh, q0:],
                            kT[ts(hh, D), g, ts(ki, 128)],
                            qT[ts(hh, D), g, ds(512 * qi + q0, w)],
                            start=True, stop=True)
                    nc.scalar.activation(
                        p4[:, ts(g, 2), q0:], sc[:, :, q0:], AF.Exp)
                    if j >= 0:
                        eng = nc.gpsimd if POLICY["masks_pool"] else nc.vector
                        for hh in range(2):
                            eng.tensor_mul(
                                out=p4[:, 2 * g + hh, q0:q0 + 128],
                                in0=p4[:, 2 * g + hh, q0:q0 + 128],
                                in1=S_sb[:])
                p4_all[(qi, ki)] = p4

            def emit_pv(tg, last_ki=None):
                # last_ki < tg leaves the accumulation groups open; a later
                # emit_pv_fin() adds the remaining k-blocks and closes them.
                tg_rel, qi = tg % 4, tg // 4
                if last_ki is None:
                    last_ki = tg
                py4 = ps_y.tile([128, HPC, D + 1], f32, tag="py")
                for h in range(HPC):
                    for ki in range(last_ki + 1):
                        nc.tensor.matmul(
                            py4[:, h, :],
                            p4_all[(qi, ki)][:, h, ts(tg_rel, 128)],
                            vaug[:, ki, h, :],
                            start=(ki == 0), stop=(ki == tg))
                py4s[tg] = py4
                pv_done[tg] = last_ki
                if not POLICY["norm_lag"] and last_ki == tg:
                    emit_norm(tg)

            def emit_pv_fin(tg):
                tg_rel, qi = tg % 4, tg // 4
                py4 = py4s[tg]
                for h in range(HPC):
                    for ki in range(pv_done[tg] + 1, tg + 1):
                        nc.tensor.matmul(
                            py4[:, h, :],
                            p4_all[(qi, ki)][:, h, ts(tg_rel, 128)],
                            vaug[:, ki, h, :],
                            start=(ki == 0), stop=(ki == tg))
                pv_done[tg] = tg

            pv_done = [None] * NT128

            def emit_norm(tg):
                py4 = py4s[tg]
                rec4 = yn.tile([128, HPC, 1], f32, tag="rec")
                nc.vector.reciprocal(rec4[:], py4[:, :, D:D + 1])
                y_n = yn.tile([128, HPC, D], bf16, tag="yn")
                nc.vector.tensor_mul(
                    out=y_n[:], in0=py4[:, :, 0:D],
                    in1=rec4.to_broadcast([128, HPC, D]))
                y_ns[tg] = y_n

            py4s = [None] * NT128
            y_ns = [None] * NT128

            def emit_transpose(tg):
                if POLICY["dma_t"]:
                    for i in range(2):
                        nc.sync.dma_start_transpose(
                            yT[:, i, ts(tg, 128)], y_ns[tg][:, ts(i, 2), :])
                    proj_q.append(tg)
                    return
                if POLICY["yt_in_big"]:
                    yTt = ps_big.tile([128, 2, 128], bf16, tag="big")
                else:
                    yTt = ps_y.tile([128, 2, 128], bf16, tag="py")
                for i in range(2):
                    nc.tensor.transpose(
                        yTt[:, i, :], y_ns[tg][:, ts(i, 2), :], Id_sb[:])
                nc.vector.tensor_copy(yT[:, :, ts(tg, 128)], yTt[:])
                proj_q.append(tg)

            def emit_proj(tg, split_drain=False):
                if POLICY["proj_merge"]:
                    pos2 = [ps_big.tile([128, 512], f32, tag="big",
                                        name=f"po_m{k}") for k in range(2)]
                    for oi in range(2):
                        for s in range(2):
                            nc.tensor.matmul(
                                pos2[oi][:], yT[:, s, ts(tg, 128)],
                                wpt_sb[:, s, ts(oi, 512)],
                                start=(s == 0), stop=(s == 1))
                    ot2 = op.tile([128, 2, 512], out_dt, tag="ot2")
                    nc.vector.tensor_copy(ot2[:, 0, :], pos2[0][:])
                    if split_drain and POLICY["drain_alt"] in (1, 2):
                        nc.scalar.activation(ot2[:, 1, :], pos2[1][:], AF.Copy)
                    else:
                        nc.vector.tensor_copy(ot2[:, 1, :], pos2[1][:])
                    nc.sync.dma_start(out[ts(tg, 128), :], ot2[:])
                    return
                if POLICY["proj_fine"]:
                    for oi in range(2):
                        ot = op.tile([128, 512], out_dt, tag="ot")
                        for q in range(2):
                            po = ps_big.tile([128, 256], f32, tag="big")
                            for s in range(2):
                                nc.tensor.matmul(
                                    po[:], yT[:, s, ts(tg, 128)],
                                    wpt_sb[:, s, ds(512 * oi + 256 * q, 256)],
                                    start=(s == 0), stop=(s == 1))
                            nc.vector.tensor_copy(ot[:, ts(q, 256)], po[:])
                        nc.sync.dma_start(
                            out[ts(tg, 128), ts(oi, 512)], ot[:])
                    return
                for oi in range(2):
                    po = ps_big.tile([128, 512], f32, tag="big")
                    for s in range(2):
                        nc.tensor.matmul(
                            po[:], yT[:, s, ts(tg, 128)],
                            wpt_sb[:, s, ts(oi, 512)],
                            start=(s == 0), stop=(s == 1))
                    ot = op.tile([128, 512], out_dt, tag="ot")
                    da = POLICY["drain_alt"]
                    act_drain = oi == 1 and (da == 1
                                             or (da == 2 and split_drain))
                    if split_drain and not da:
                        # halve the drain latency: DVE + ACT in parallel
                        nc.vector.tensor_copy(ot[:, :256], po[:, :256])
                        nc.scalar.activation(
                            ot[:, 256:], po[:, 256:], AF.Copy)
                    elif act_drain:
                        nc.scalar.activation(ot[:], po[:], AF.Copy)
                    else:
                        nc.vector.tensor_copy(ot[:], po[:])
                    nc.sync.dma_start(out[ts(tg, 128), ts(oi, 512)], ot[:])

            # -------- software-pipelined emission --------
            # Stage lags (in pipeline slots): pv(k) | norm(k-1) |
            # transpose(k-2). The lag keeps every engine's in-order stream
            # free of head-of-line waits: by the time a stage is emitted its
            # producer ran a whole slot earlier. proj tiles are held in a
            # queue and spent as PE filler inside ACT-bound score regions
            # (mainly stripe 3) and the tail.
            filler = deque()   # pending PE-heavy units (qkv chains, late v)
            proj_q = deque()   # proj tiles ready to emit
            state = {"pv": 0}

            def advance_pipeline(upto, spend_proj=False):
                nl = POLICY["norm_lag"]
                sp = POLICY["pv_split"]
                while state["pv"] <= min(upto, NT128 - 1):
                    tg = state["pv"]
                    if sp:
                        if tg >= 1:
                            emit_pv_fin(tg - 1)
                        if tg >= 2:
                            emit_norm(tg - 2)
                        if tg >= 3:
                            emit_transpose(tg - 3)
                        if spend_proj and proj_q:
                            emit_proj(proj_q.popleft())
                        emit_pv(tg, last_ki=tg - 1)
                    else:
                        if nl and tg >= 1:
                            emit_norm(tg - 1)
                        if tg >= 1 + nl:
                            emit_transpose(tg - 1 - nl)
                        if spend_proj and proj_q:
                            emit_proj(proj_q.popleft())
                        emit_pv(tg)
                    state["pv"] += 1

            for ti in range(NT512):
                ensure_xt_dma(ti)
                for pf in range(1, POLICY["xt_prefetch"] + 1):
                    ensure_xt_dma(ti + pf)
                fast0 = ti == 0 and POLICY["fast_start"]
                if ti == 0:
                    emit_qk_units(0)
                # v chains: stripes 0-2 inline; stripe 3 deferred as filler.
                # Under fast_start stripe 0's v waits the f32 x reload, so it
                # moves after the (bf16-fed) stripe-0 scores.
                defer_v = (ti == 3 and POLICY["v3_defer"]) or \
                          (ti >= 1 and POLICY["v_defer_all"])
                if fast0:
                    pass
                elif defer_v:
                    # v feeds this stripe's own pv (diag region), so these
                    # units go to the FRONT of the filler queue.
                    for tj in range(3, -1, -1):
                        filler.appendleft((emit_v_chain, (ti, tj)))
                else:
                    for tj in range(4):
                        emit_v_chain(ti, tj)
                # next stripe's q/k chains become filler inside our scores
                if ti + 1 < NT512:
                    for unit in qk_units(ti + 1):
                        filler.append(unit)
                # finish previous stripe's last tiles (their exps are done):
                # either as one burst here, or spread into the score loop as
                # filler units so ACT gets fed sooner.
                if POLICY["adv_fill"] and ti > 0:
                    units = [(advance_pipeline, (m,))
                             for m in range(state["pv"], 4 * ti)]
                    for u in reversed(units):
                        filler.appendleft(u)
                else:
                    advance_pipeline(4 * ti - 1)

                nk = 4 * ti + 4
                spend = POLICY["spend_proj"]
                nfill = 1 + (POLICY["double_fill"] and ti == 3)
                for ki in range(nk):
                    def _slot_work():
                        if fast0:
                            return
                        if ki - 1 >= 4 * ti:
                            advance_pipeline(
                                ki - 1,
                                spend_proj=(spend == "all"
                                            or (spend in ("s3", "s23")
                                                and ti == 3)
                                            or (spend == "s23" and ti == 2)))
                        else:
                            for _ in range(nfill):
                                if filler:
                                    fn, args = filler.popleft()
                                    fn(*args)
                                elif proj_q:
                                    emit_proj(proj_q.popleft())
                    sgroups = ((1,) if (ti == 0 and POLICY["s0_gsplit"])
                               else (0, 1))
                    if POLICY["slot_swap"]:
                        _slot_work()
                        emit_scores(ti, ki, groups=sgroups)
                    else:
                        emit_scores(ti, ki, groups=sgroups)
                        _slot_work()
                if fast0:
                    for tj in range(4):
                        emit_v_chain(0, tj)
                    advance_pipeline(2)
                # flush leftover qkv filler before the next stripe needs it
                while filler:
                    fn, args = filler.popleft()
                    fn(*args)

            def emit_tail_fine():
                # tile 15: per-slab norm -> transpose -> yT copy interleaved
                # with the proj contraction chain, to shorten the serial tail
                tg = NT128 - 1
                py4 = py4s[tg]
                y_n = yn.tile([128, HPC, D], bf16, tag="yn")
                yTt = ps_y.tile([128, 2, 128], bf16, tag="py")
                pos = [ps_big.tile([128, 512], f32, tag="big", name=f"po_t{k}")
                       for k in range(2)]
                for s in range(2):
                    rec2 = yn.tile([128, 2, 1], f32, tag="rec")
                    nc.vector.reciprocal(
                        rec2[:], py4[:, ts(s, 2), D:D + 1])
                    nc.vector.tensor_mul(
                        out=y_n[:, ts(s, 2), :], in0=py4[:, ts(s, 2), 0:D],
                        in1=rec2.to_broadcast([128, 2, D]))
                    nc.tensor.transpose(
                        yTt[:, s, :], y_n[:, ts(s, 2), :], Id_sb[:])
                    nc.vector.tensor_copy(
                        yT[:, s, ts(tg, 128)], yTt[:, s, :])
                    for oi in range(2):
                        nc.tensor.matmul(
                            pos[oi][:], yT[:, s, ts(tg, 128)],
                            wpt_sb[:, s, ts(oi, 512)],
                            start=(s == 0), stop=(s == 1))
                # drain both halves into one staging tile and write the final
                # row-block with a single DMA: one fewer HWDGE+DGE+sem chain
                # on the kernel's critical tail.
                ot2 = op.tile([128, 2, 512], out_dt, tag="ot2")
                nc.vector.tensor_copy(ot2[:, 0, :], pos[0][:])
                if POLICY["drain_alt"] in (1, 2):
                    nc.scalar.activation(ot2[:, 1, :], pos[1][:], AF.Copy)
                else:
                    nc.vector.tensor_copy(ot2[:, 1, :], pos[1][:])
                nc.sync.dma_start(out[ts(tg, 128), :], ot2[:])

            advance_pipeline(NT128 - 1)
            if POLICY["pv_split"]:
                emit_pv_fin(NT128 - 1)
                emit_norm(NT128 - 2)
                emit_transpose(NT128 - 3)
                emit_norm(NT128 - 1)
                emit_transpose(NT128 - 2)
            elif POLICY["norm_lag"]:
                if POLICY["tail_fine"]:
                    emit_transpose(NT128 - 2)
                    while proj_q:
                        emit_proj(proj_q.popleft(),
                                  split_drain=POLICY["drain_alt"] == 2)
                    emit_tail_fine()
                else:
                    emit_norm(NT128 - 1)
                    emit_transpose(NT128 - 2)
            if not (POLICY["norm_lag"] and POLICY["tail_fine"]):
                emit_transpose(NT128 - 1)
                while proj_q:
                    emit_proj(proj_q.popleft(),
                              split_drain=bool(POLICY["tail_split"])
                              or POLICY["drain_alt"] == 2)

    nc.compile()
    return nc


def _get_compiled():
    global _COMPILED
    if _COMPILED is None:
        _COMPILED = _build()
    return _COMPILED


def _host_prep(x, W_attn, b_attn, W_proj, b_proj):
    scale = 1.0 / np.sqrt(np.float32(D))
    xTb = [np.ascontiguousarray(x[b].T).astype(np.float32) for b in range(B)]
    Sm = (np.arange(128, dtype=np.int32)[None, :]
          >= np.arange(128, dtype=np.int32)[:, None]).astype(np.float32)
    Idm = np.eye(128, dtype=np.float32)
    in_maps = []
    for c in range(N_CORES):
        b, g = divmod(c, 4)
        ch = slice(CH * g, CH * (g + 1))
        Wq = W_attn[ch]
        Wk = W_attn[C:][ch] * scale
        Wv = W_attn[2 * C:][ch]
        wt_c = np.ascontiguousarray(
            np.concatenate([Wq, Wk, Wv], axis=0).T).astype(np.float32)
        bq = b_attn[ch]
        bk = b_attn[C:][ch] * scale
        bv = b_attn[2 * C:][ch]
        bqk_c = np.ascontiguousarray(
            np.concatenate([bq, bk]).reshape(4, 128).T).astype(np.float32)
        bvb_c = np.ascontiguousarray(
            np.broadcast_to(bv[None, :].reshape(1, HPC, D),
                            (128, HPC, D))).astype(np.float32)
        wpt_c = np.ascontiguousarray(W_proj[:, ch].T).astype(np.float32)
        if POLICY["qkv_bf16"]:
            import ml_dtypes
            xT_in = xTb[b].astype(ml_dtypes.bfloat16)
            wt_in = wt_c.astype(ml_dtypes.bfloat16)
        else:
            xT_in, wt_in = xTb[b], wt_c
        im = {
            "xT": xT_in,
            "wt": wt_in,
        }
        if POLICY["xw_merge"]:
            import ml_dtypes
            im["xw0"] = np.ascontiguousarray(np.concatenate(
                [xTb[b][:, :512], wt_c[:, :512]],
                axis=1)).astype(ml_dtypes.bfloat16)
        im.update({
            "wpt": wpt_c,
            "bqk": bqk_c,
            "bvb": bvb_c,
            "Sm": Sm,
            "Idm": Idm,
        })
        if POLICY["fast_start"]:
            import ml_dtypes
            im["x0b"] = np.ascontiguousarray(
                xTb[b][:, :512]).astype(ml_dtypes.bfloat16)
            im["wqkb"] = np.ascontiguousarray(
                wt_c[:, :512]).astype(ml_dtypes.bfloat16)
        in_maps.append(im)
    return in_maps


def kernel(x, W_attn, b_attn, W_proj, b_proj):
    x = np.asarray(x, dtype=np.float32)
    W_attn = np.asarray(W_attn, dtype=np.float32)
    b_attn = np.asarray(b_attn, dtype=np.float32)
    W_proj = np.asarray(W_proj, dtype=np.float32)
    b_proj = np.asarray(b_proj, dtype=np.float32)

    nc = _get_compiled()
    in_maps = _host_prep(x, W_attn, b_attn, W_proj, b_proj)
    res = run_bass_kernel_spmd(nc, in_maps, core_ids=list(range(N_CORES)))

    out = np.empty((B, T, C), dtype=np.float32)
    for b in range(B):
        acc = np.asarray(res.results[4 * b]["out_partial"],
                         dtype=np.float32).copy()
        for g in range(1, 4):
            acc += np.asarray(res.results[4 * b + g]["out_partial"],
                              dtype=np.float32)
        out[b] = acc + b_proj
    return out



# revision 2
# speedup vs baseline: 1.0505x; 1.0505x over previous
"""Causal self-attention on 8 NeuronCores (Bass/Tile).

Sharding: tensor-parallel over heads x data-parallel over batch.
  core c -> batch b = c//4, heads 4g..4g+3 where g = c%4.
Each core computes q,k,v for its 4 heads (over its batch's 2048 tokens),
causal softmax attention, and the partial output projection over its 256
head-channels. Host sums the 4 partials per batch and adds b_proj.

v3 design: fp8(e4m3) DoubleRow matmuls for the q/k projection chains and
the score matmuls (cost model: DoubleRow fp8 = 0.5 cyc/row with 2x128
contraction per instruction -> 4x cheaper qk projection, 2x cheaper
scores). Numerics (measured vs f32 reference): ~1.65e-2 max-rel, under
the 2e-2 gate. v/pv/proj stay bf16 (fp8 there fails the gate).

Layout for fp8 scores: per head the contraction is d=64, split as
[32 partitions x 2 DoubleRow sub-rows]. q/k are stored as two tile sets:
  tile A: heads 0,1 at partition offsets 0,32 (the direct drain target)
  tile B: heads 2,3, DMA-shifted from A's partitions 64-127 down to 0-63
(PE matmuls with lhsT/rhs partition base 64/96 fail BIR/runtime; SBUF->
SBUF DMA moves across partitions instead). The qk psum chains emit the
channel order c = 64*(p//32) + 32*i + p%32 via host-side W column
permutation, so each drain stays partition-aligned. Drains are DVE
tensor_scalar (psum * QS/(XS*WS) + QS*bias -> fp8), with exp scale
1/QS^2 folded into the ACT activation.

With PE cut to ~65 us the Activation engine (exp: ~58 us of elements +
per-instr bubbles) becomes the critical engine; emission keeps ACT fed:
scores are emitted just-in-time ahead of their exps, and all bf16 PE
work (v chains, pv, transpose, proj) + qk chains ride as filler between
score slots. Masks run on GPSIMD(Pool), off the DVE/ACT critical paths.
"""

import os
import sys

for _p in ("/opt/trn_rl_repo", "/opt/pypackages"):
    if os.path.isdir(_p) and _p not in sys.path:
        sys.path.append(_p)

import numpy as np

import concourse.bass as bass
import concourse.tile as tile
import concourse.mybir as mybir
from concourse import bacc
from concourse.bass_utils import run_bass_kernel_spmd

B, T, C = 2, 2048, 1024
H = 16            # total heads
D = 64            # head dim
HPC = 4           # heads per core
CH = HPC * D      # 256 channels per core
N_CORES = 8

f32 = mybir.dt.float32
bf16 = mybir.dt.bfloat16
fp8 = mybir.dt.float8e4
ts = bass.ts
ds = bass.ds
AF = mybir.ActivationFunctionType
ALU = mybir.AluOpType
PM = mybir.MatmulPerfMode

XS = 8.0    # fp8 x pre-scale
WS = 64.0   # fp8 W pre-scale
QS = 2.0    # stored q/k fp8 scale
DRAIN_S = float(QS / (XS * WS))
EXP_S = float(1.0 / (QS * QS))

_COMPILED = None

POLICY = {
    "norm_lag": int(os.environ.get("K_NORM_LAG", "1")),
    "spend_proj": os.environ.get("K_SPEND_PROJ", "s3"),  # none|s3|s23|all
    "sc_bufs": int(os.environ.get("K_SC_BUFS", "2")),
    "big_bufs": int(os.environ.get("K_BIG_BUFS", "2")),
    "masks_pool": int(os.environ.get("K_MASKS_POOL", "1")),
    "v3_defer": int(os.environ.get("K_V3_DEFER", "1")),
    "out_bf16": int(os.environ.get("K_OUT_BF16", "1")),
    "adv_fill": int(os.environ.get("K_ADV_FILL", "1")),
    "py_bufs": int(os.environ.get("K_PY_BUFS", "2")),
    "xt_prefetch": int(os.environ.get("K_XT_PREFETCH", "1")),
    "drain_alt": int(os.environ.get("K_DRAIN_ALT", "2")),
    "tail_fine": int(os.environ.get("K_TAIL_FINE", "1")),
    "proj_merge": int(os.environ.get("K_PROJ_MERGE", "1")),
    "v_defer_all": int(os.environ.get("K_V_DEFER_ALL", "1")),
    "double_fill": int(os.environ.get("K_DOUBLE_FILL", "0")),
    "slot_swap": int(os.environ.get("K_SLOT_SWAP", "0")),
    "fp8_scores": int(os.environ.get("K_FP8_SCORES", "1")),  # fallback knob
    "warm_exp": int(os.environ.get("K_WARM_EXP", "1")),
}


def _build():
    nc = bacc.Bacc("TRN2", target_bir_lowering=False, debug=False,
                   num_devices=N_CORES)

    # DRAM inputs (host-prepped layouts)
    x8 = nc.dram_tensor("x8", [128, 4, 2, T], fp8, kind="ExternalInput").ap()
    wqk8 = nc.dram_tensor("wqk8", [128, 4, 2, 512], fp8,
                          kind="ExternalInput").ap()
    xT = nc.dram_tensor("xT", [C, T], bf16, kind="ExternalInput").ap()
    wv = nc.dram_tensor("wv", [128, 8, CH], bf16, kind="ExternalInput").ap()
    wpt = nc.dram_tensor("wpt", [CH, C], f32, kind="ExternalInput").ap()
    bqk = nc.dram_tensor("bqk", [128, 4], f32, kind="ExternalInput").ap()
    bvb = nc.dram_tensor("bvb", [128, HPC, D], f32, kind="ExternalInput").ap()
    Sm = nc.dram_tensor("Sm", [128, 128], f32, kind="ExternalInput").ap()
    Idm = nc.dram_tensor("Idm", [128, 128], f32, kind="ExternalInput").ap()
    out_dt = bf16 if POLICY["out_bf16"] else f32
    out = nc.dram_tensor("out_partial", [T, C], out_dt,
                         kind="ExternalOutput").ap()

    NT512 = T // 512          # 4   512-token stripes
    NT128 = T // 128          # 16  128-token tiles
    NC128 = C // 128          # 8   contraction tiles (bf16 v path)

    with tile.TileContext(nc) as tc:
        with tc.tile_pool(name="consts", bufs=1) as consts, \
             tc.tile_pool(name="qkv", bufs=1) as qkv, \
             tc.tile_pool(name="x8p",
                          bufs=POLICY["xt_prefetch"] + 1) as x8p, \
             tc.tile_pool(name="xp",
                          bufs=POLICY["xt_prefetch"] + 1) as xp, \
             tc.tile_pool(name="pp", bufs=17) as pp, \
             tc.tile_pool(name="yn", bufs=2) as yn, \
             tc.tile_pool(name="op", bufs=3) as op, \
             tc.tile_pool(name="ps_s", bufs=POLICY["sc_bufs"],
                          space="PSUM") as ps_s, \
             tc.tile_pool(name="ps_y", bufs=POLICY["py_bufs"],
                          space="PSUM") as ps_y, \
             tc.tile_pool(name="ps_big", bufs=POLICY["big_bufs"],
                          space="PSUM") as ps_big:

            # ---- startup DMAs: wqk8 + x8 stripe 0 chunked by j so the
            #      first DR chain starts after ~1.3us ----
            wqk_sb = consts.tile([128, 4, 2, 512], fp8)
            x8t0 = x8p.tile([128, 4, 2, 512], fp8, tag="x8t")
            for j in range(4):
                nc.sync.dma_start(wqk_sb[:, j], wqk8[:, j])
                nc.sync.dma_start(x8t0[:, j], x8[:, j, :, ts(0, 512)])
            bqk_sb = consts.tile([128, 4], f32)
            nc.sync.dma_start(bqk_sb[:], bqk)

            # dummy exp: pulls LoadActFuncSet into the startup DMA window
            if POLICY["warm_exp"]:
                warm = consts.tile([128, 1], f32)
                nc.vector.memset(warm[:], 0.0)
                warm_o = consts.tile([128, 1], bf16)
                nc.scalar.activation(warm_o[:], warm[:], AF.Exp)

            # bf16 x (v path) + remaining consts
            xT_r = xT.rearrange("(o p) t -> p o t", p=128)
            xt0 = xp.tile([128, NC128, 512], bf16, tag="xt")
            nc.sync.dma_start(xt0[:], xT_r[:, :, ts(0, 512)])
            wv_sb = consts.tile([128, 8, CH], bf16)
            nc.sync.dma_start(wv_sb[:], wv)
            bvb_sb = consts.tile([128, HPC, D], f32)
            nc.sync.dma_start(bvb_sb[:], bvb)
            S_f = consts.tile([128, 128], f32)
            nc.sync.dma_start(S_f[:], Sm)
            Id_f = consts.tile([128, 128], f32)
            nc.sync.dma_start(Id_f[:], Idm)
            wpt_f = consts.tile([128, 2, C], f32)
            nc.sync.dma_start(
                wpt_f[:], wpt.rearrange("(s p) o -> p s o", p=128))

            S_sb = consts.tile([128, 128], bf16)
            nc.gpsimd.tensor_copy(S_sb[:], S_f[:])
            Id_sb = consts.tile([128, 128], bf16)
            nc.gpsimd.tensor_copy(Id_sb[:], Id_f[:])
            wpt_sb = consts.tile([128, 2, C], bf16)
            nc.gpsimd.tensor_copy(wpt_sb[:], wpt_f[:])

            # ---- persistent activations ----
            # q/k fp8: tile A holds heads 0,1 (parts 0..63) as drained;
            # B gets heads 2,3 DMA-shifted from A's parts 64..127.
            qT8a = qkv.tile([128, 2, T], fp8)
            qT8b = qkv.tile([128, 2, T], fp8)
            kT8a = qkv.tile([128, 2, T], fp8)
            kT8b = qkv.tile([128, 2, T], fp8)
            vaug = qkv.tile([128, NT128, HPC, D + 1], bf16)  # [kt,ki,h,d|1]
            yT = qkv.tile([128, 2, T], bf16)

            nc.vector.memset(vaug[:, :, :, D:D + 1], 1.0)

            # ---------------- emission helpers ----------------
            from collections import deque

            x8_tiles = {0: x8t0}
            xt_tiles = {0: xt0}

            def ensure_xt_dma(ti):
                if ti < NT512 and ti not in x8_tiles:
                    x8t = x8p.tile([128, 4, 2, 512], fp8, tag="x8t")
                    nc.sync.dma_start(x8t[:], x8[:, :, :, ts(ti, 512)])
                    x8_tiles[ti] = x8t
                if ti < NT512 and ti not in xt_tiles:
                    xt = xp.tile([128, NC128, 512], bf16, tag="xt")
                    nc.sync.dma_start(xt[:], xT_r[:, :, ts(ti, 512)])
                    xt_tiles[ti] = xt

            # chain cc: 0=q,i0  1=q,i1  2=k,i0  3=k,i1
            def emit_qk_chain(ti, cc):
                x8t = x8_tiles[ti]
                st = qT8a if cc < 2 else kT8a
                i = cc % 2
                ps = ps_big.tile([128, 512], f32, tag="big")
                for j in range(4):
                    nc.tensor.matmul(
                        ps[:], wqk_sb[:, j, :, ts(cc, 128)], x8t[:, j],
                        start=(j == 0), stop=(j == 3),
                        perf_mode=PM.DoubleRow)
                nc.vector.tensor_scalar(
                    st[:, i, ts(ti, 512)], ps[:], DRAIN_S,
                    bqk_sb[:, cc:cc + 1], op0=ALU.mult, op1=ALU.add)

            def emit_qk_shift(ti, qk):
                # move heads 2,3 (parts 64..127) down to parts 0..63 of B
                a, b = (qT8a, qT8b) if qk == 0 else (kT8a, kT8b)
                nc.sync.dma_start(b[0:64, :, ts(ti, 512)],
                                  a[ds(64, 64), :, ts(ti, 512)])

            def qk_units(ti):
                return ([(emit_qk_chain, (ti, cc)) for cc in (0, 2)]
                        + [(emit_qk_chain, (ti, cc)) for cc in (1, 3)]
                        + [(emit_qk_shift, (ti, 0)), (emit_qk_shift, (ti, 1))])

            def emit_v_chain(ti, tj):
                xt = xt_tiles[ti]
                pv = ps_big.tile([128, HPC, D], f32, tag="big")
                for ci in range(NC128):
                    nc.tensor.matmul(
                        pv[:, :, :], xt[:, ci, ts(tj, 128)],
                        wv_sb[:, ci, :],
                        start=(ci == 0), stop=(ci == NC128 - 1))
                nc.vector.tensor_add(
                    out=vaug[:, 4 * ti + tj, :, 0:D],
                    in0=pv[:, :, :], in1=bvb_sb[:])

            p4_all = {}  # (stripe, ki) -> p4 tile

            def emit_scores(qi, ki, groups=(0, 1)):
                j = ki - 4 * qi
                q0 = max(0, 128 * j)
                w = 512 - q0
                if (qi, ki) in p4_all:
                    p4 = p4_all[(qi, ki)]
                else:
                    p4 = pp.tile([128, HPC, 512], bf16, tag="p4")
                for g in groups:
                    KT = kT8a if g == 0 else kT8b
                    QT = qT8a if g == 0 else qT8b
                    sc = ps_s.tile([128, 2, 512], f32, tag="sc")
                    for hh in range(2):
                        nc.tensor.matmul(
                            sc[:, hh, q0:],
                            KT[ts(hh, 32), :, ts(ki, 128)],
                            QT[ts(hh, 32), :, ds(512 * qi + q0, w)],
                            start=True, stop=True,
                            perf_mode=PM.DoubleRow,
                            tile_position=(32 * hh, 0))
                    nc.scalar.activation(
                        p4[:, ts(g, 2), q0:], sc[:, :, q0:], AF.Exp,
                        scale=EXP_S)
                    if j >= 0:
                        eng = nc.gpsimd if POLICY["masks_pool"] else nc.vector
                        for hh in range(2):
                            eng.tensor_mul(
                                out=p4[:, 2 * g + hh, q0:q0 + 128],
                                in0=p4[:, 2 * g + hh, q0:q0 + 128],
                                in1=S_sb[:])
                p4_all[(qi, ki)] = p4

            def emit_pv(tg):
                tg_rel, qi = tg % 4, tg // 4
                py4 = ps_y.tile([128, HPC, D + 1], f32, tag="py")
                for h in range(HPC):
                    for ki in range(tg + 1):
                        nc.tensor.matmul(
                            py4[:, h, :],
                            p4_all[(qi, ki)][:, h, ts(tg_rel, 128)],
                            vaug[:, ki, h, :],
                            start=(ki == 0), stop=(ki == tg))
                py4s[tg] = py4
                if not POLICY["norm_lag"]:
                    emit_norm(tg)

            def emit_norm(tg):
                py4 = py4s[tg]
                rec4 = yn.tile([128, HPC, 1], f32, tag="rec")
                nc.vector.reciprocal(rec4[:], py4[:, :, D:D + 1])
                y_n = yn.tile([128, HPC, D], bf16, tag="yn")
                nc.vector.tensor_mul(
                    out=y_n[:], in0=py4[:, :, 0:D],
                    in1=rec4.to_broadcast([128, HPC, D]))
                y_ns[tg] = y_n

            py4s = [None] * NT128
            y_ns = [None] * NT128

            def emit_transpose(tg):
                yTt = ps_y.tile([128, 2, 128], bf16, tag="py")
                for i in range(2):
                    nc.tensor.transpose(
                        yTt[:, i, :], y_ns[tg][:, ts(i, 2), :], Id_sb[:])
                nc.vector.tensor_copy(yT[:, :, ts(tg, 128)], yTt[:])
                proj_q.append(tg)

            def emit_proj(tg, split_drain=False):
                if POLICY["proj_merge"]:
                    pos2 = [ps_big.tile([128, 512], f32, tag="big",
                                        name=f"po_m{k}") for k in range(2)]
                    for oi in range(2):
                        for s in range(2):
                            nc.tensor.matmul(
                                pos2[oi][:], yT[:, s, ts(tg, 128)],
                                wpt_sb[:, s, ts(oi, 512)],
                                start=(s == 0), stop=(s == 1))
                    ot2 = op.tile([128, 2, 512], out_dt, tag="ot2")
                    nc.vector.tensor_copy(ot2[:, 0, :], pos2[0][:])
                    if split_drain and POLICY["drain_alt"] in (1, 2):
                        nc.scalar.activation(ot2[:, 1, :], pos2[1][:], AF.Copy)
                    else:
                        nc.vector.tensor_copy(ot2[:, 1, :], pos2[1][:])
                    nc.sync.dma_start(out[ts(tg, 128), :], ot2[:])
                    return
                for oi in range(2):
                    po = ps_big.tile([128, 512], f32, tag="big")
                    for s in range(2):
                        nc.tensor.matmul(
                            po[:], yT[:, s, ts(tg, 128)],
                            wpt_sb[:, s, ts(oi, 512)],
                            start=(s == 0), stop=(s == 1))
                    ot = op.tile([128, 512], out_dt, tag="ot")
                    nc.vector.tensor_copy(ot[:], po[:])
                    nc.sync.dma_start(out[ts(tg, 128), ts(oi, 512)], ot[:])

            # -------- software-pipelined emission --------
            filler = deque()   # pending PE-heavy units
            proj_q = deque()   # proj tiles ready to emit
            state = {"pv": 0}

            def advance_pipeline(upto, spend_proj=False):
                nl = POLICY["norm_lag"]
                while state["pv"] <= min(upto, NT128 - 1):
                    tg = state["pv"]
                    if nl and tg >= 1:
                        emit_norm(tg - 1)
                    if tg >= 1 + nl:
                        emit_transpose(tg - 1 - nl)
                    if spend_proj and proj_q:
                        emit_proj(proj_q.popleft())
                    emit_pv(tg)
                    state["pv"] += 1

            for ti in range(NT512):
                ensure_xt_dma(ti)
                for pf in range(1, POLICY["xt_prefetch"] + 1):
                    ensure_xt_dma(ti + pf)
                if ti == 0:
                    for fn, args in qk_units(0):
                        fn(*args)
                defer_v = (ti == 3 and POLICY["v3_defer"]) or \
                          (ti >= 1 and POLICY["v_defer_all"])
                if defer_v:
                    for tj in range(3, -1, -1):
                        filler.appendleft((emit_v_chain, (ti, tj)))
                else:
                    for tj in range(4):
                        emit_v_chain(ti, tj)
                if ti + 1 < NT512:
                    for unit in qk_units(ti + 1):
                        filler.append(unit)
                if POLICY["adv_fill"] and ti > 0:
                    units = [(advance_pipeline, (m,))
                             for m in range(state["pv"], 4 * ti)]
                    for u in reversed(units):
                        filler.appendleft(u)
                else:
                    advance_pipeline(4 * ti - 1)

                nk = 4 * ti + 4
                spend = POLICY["spend_proj"]
                nfill = 1 + (POLICY["double_fill"] and ti == 3)
                for ki in range(nk):
                    def _slot_work():
                        if ki - 1 >= 4 * ti:
                            advance_pipeline(
                                ki - 1,
                                spend_proj=(spend == "all"
                                            or (spend in ("s3", "s23")
                                                and ti == 3)
                                            or (spend == "s23" and ti == 2)))
                        else:
                            for _ in range(nfill):
                                if filler:
                                    fn, args = filler.popleft()
                                    fn(*args)
                                elif proj_q:
                                    emit_proj(proj_q.popleft())
                    if POLICY["slot_swap"]:
                        _slot_work()
                        emit_scores(ti, ki)
                    else:
                        emit_scores(ti, ki)
                        _slot_work()
                while filler:
                    fn, args = filler.popleft()
                    fn(*args)

            def emit_tail_fine():
                # tile 15: per-slab norm -> transpose -> yT copy interleaved
                # with the proj contraction chain, to shorten the serial tail
                tg = NT128 - 1
                py4 = py4s[tg]
                y_n = yn.tile([128, HPC, D], bf16, tag="yn")
                yTt = ps_y.tile([128, 2, 128], bf16, tag="py")
                pos = [ps_big.tile([128, 512], f32, tag="big", name=f"po_t{k}")
                       for k in range(2)]
                for s in range(2):
                    rec2 = yn.tile([128, 2, 1], f32, tag="rec")
                    nc.vector.reciprocal(
                        rec2[:], py4[:, ts(s, 2), D:D + 1])
                    nc.vector.tensor_mul(
                        out=y_n[:, ts(s, 2), :], in0=py4[:, ts(s, 2), 0:D],
                        in1=rec2.to_broadcast([128, 2, D]))
                    nc.tensor.transpose(
                        yTt[:, s, :], y_n[:, ts(s, 2), :], Id_sb[:])
                    nc.vector.tensor_copy(
                        yT[:, s, ts(tg, 128)], yTt[:, s, :])
                    for oi in range(2):
                        nc.tensor.matmul(
                            pos[oi][:], yT[:, s, ts(tg, 128)],
                            wpt_sb[:, s, ts(oi, 512)],
                            start=(s == 0), stop=(s == 1))
                ot2 = op.tile([128, 2, 512], out_dt, tag="ot2")
                nc.vector.tensor_copy(ot2[:, 0, :], pos[0][:])
                if POLICY["drain_alt"] in (1, 2):
                    nc.scalar.activation(ot2[:, 1, :], pos[1][:], AF.Copy)
                else:
                    nc.vector.tensor_copy(ot2[:, 1, :], pos[1][:])
                nc.sync.dma_start(out[ts(tg, 128), :], ot2[:])

            advance_pipeline(NT128 - 1)
            if POLICY["norm_lag"]:
                if POLICY["tail_fine"]:
                    emit_transpose(NT128 - 2)
                    while proj_q:
                        emit_proj(proj_q.popleft(),
                                  split_drain=POLICY["drain_alt"] == 2)
                    emit_tail_fine()
                else:
                    emit_norm(NT128 - 1)
                    emit_transpose(NT128 - 2)
            if not (POLICY["norm_lag"] and POLICY["tail_fine"]):
                emit_transpose(NT128 - 1)
                while proj_q:
                    emit_proj(proj_q.popleft(),
                              split_drain=POLICY["drain_alt"] == 2)

    nc.compile()
    return nc


def _get_compiled():
    global _COMPILED
    if _COMPILED is None:
        _COMPILED = _build()
    return _COMPILED


def _host_prep(x, W_attn, b_attn, W_proj, b_proj):
    import ml_dtypes
    scale = 1.0 / np.sqrt(np.float32(D))
    xTb = [np.ascontiguousarray(x[b].T).astype(np.float32) for b in range(B)]
    Sm = (np.arange(128, dtype=np.int32)[None, :]
          >= np.arange(128, dtype=np.int32)[:, None]).astype(np.float32)
    Idm = np.eye(128, dtype=np.float32)

    # channel order for a qk chain with sub-row i: psum partition p' holds
    # local channel c = 64*(p'//32) + 32*i + (p'%32)
    pp_ = np.arange(128)
    c_of_p = {i: 64 * (pp_ // 32) + 32 * i + (pp_ % 32) for i in (0, 1)}

    in_maps = []
    for c in range(N_CORES):
        b, g = divmod(c, 4)
        ch = slice(CH * g, CH * (g + 1))
        Wq = W_attn[ch]                    # [256, C]
        Wk = W_attn[C:][ch] * scale
        Wv = W_attn[2 * C:][ch]
        bq = b_attn[ch]
        bk = b_attn[C:][ch] * scale
        bv = b_attn[2 * C:][ch]

        # x8: [128, 4(j), 2(i), T] = xT[(2j+i)*128+p, t] * XS
        x8_c = np.ascontiguousarray(
            (xTb[b].reshape(4, 2, 128, T).transpose(2, 0, 1, 3) * XS)
        ).astype(ml_dtypes.float8_e4m3)

        # wqk8: [128(p), 4(j), 2(i_row), 512(4 chains x 128 cols)]
        # chain cc: 0=q,i0 1=q,i1 2=k,i0 3=k,i1; col p' -> channel c_of_p
        wqk = np.empty((C, 4, 128), dtype=np.float32)  # [row, chain, col]
        wqk[:, 0, :] = Wq[c_of_p[0]].T
        wqk[:, 1, :] = Wq[c_of_p[1]].T
        wqk[:, 2, :] = Wk[c_of_p[0]].T
        wqk[:, 3, :] = Wk[c_of_p[1]].T
        wqk = wqk.reshape(4, 2, 128, 4 * 128).transpose(2, 0, 1, 3)
        wqk8_c = np.ascontiguousarray(wqk * WS).astype(ml_dtypes.float8_e4m3)

        # bqk: [128, 4] = QS * bias[channel(p', chain)]
        bqk_c = np.empty((128, 4), dtype=np.float32)
        bqk_c[:, 0] = QS * bq[c_of_p[0]]
        bqk_c[:, 1] = QS * bq[c_of_p[1]]
        bqk_c[:, 2] = QS * bk[c_of_p[0]]
        bqk_c[:, 3] = QS * bk[c_of_p[1]]

        wv_c = np.ascontiguousarray(
            Wv.T.reshape(8, 128, CH).transpose(1, 0, 2)
        ).astype(ml_dtypes.bfloat16)
        bvb_c = np.ascontiguousarray(
            np.broadcast_to(bv[None, :].reshape(1, HPC, D),
                            (128, HPC, D))).astype(np.float32)
        wpt_c = np.ascontiguousarray(W_proj[:, ch].T).astype(np.float32)

        im = {
            "x8": x8_c,
            "wqk8": wqk8_c,
            "xT": xTb[b].astype(ml_dtypes.bfloat16),
            "wv": wv_c,
            "wpt": wpt_c,
            "bqk": bqk_c,
            "bvb": bvb_c,
            "Sm": Sm,
            "Idm": Idm,
        }
        in_maps.append(im)
    return in_maps


def kernel(x, W_attn, b_attn, W_proj, b_proj):
    x = np.asarray(x, dtype=np.float32)
    W_attn = np.asarray(W_attn, dtype=np.float32)
    b_attn = np.asarray(b_attn, dtype=np.float32)
    W_proj = np.asarray(W_proj, dtype=np.float32)
    b_proj = np.asarray(b_proj, dtype=np.float32)

    nc = _get_compiled()
    in_maps = _host_prep(x, W_attn, b_attn, W_proj, b_proj)
    res = run_bass_kernel_spmd(nc, in_maps, core_ids=list(range(N_CORES)))

    out = np.empty((B, T, C), dtype=np.float32)
    for b in range(B):
        acc = np.asarray(res.results[4 * b]["out_partial"],
                         dtype=np.float32).copy()
        for g in range(1, 4):
            acc += np.asarray(res.results[4 * b + g]["out_partial"],
                              dtype=np.float32)
        out[b] = acc + b_proj
    return out


# revision 16
# speedup vs baseline: 1.0691x; 1.0177x over previous
"""Causal self-attention on 8 NeuronCores (Bass/Tile).

Sharding: tensor-parallel over heads x data-parallel over batch.
  core c -> batch b = c//4, heads 4g..4g+3 where g = c%4.
Each core computes q,k,v for its 4 heads (over its batch's 2048 tokens),
causal softmax attention, and the partial output projection over its 256
head-channels. Host sums the 4 partials per batch and adds b_proj.

v3 design: fp8(e4m3) DoubleRow matmuls for the q/k projection chains and
the score matmuls (cost model: DoubleRow fp8 = 0.5 cyc/row with 2x128
contraction per instruction -> 4x cheaper qk projection, 2x cheaper
scores). Numerics (measured vs f32 reference): ~1.65e-2 max-rel, under
the 2e-2 gate. v/pv/proj stay bf16 (fp8 there fails the gate).

Layout for fp8 scores: per head the contraction is d=64, split as
[32 partitions x 2 DoubleRow sub-rows]. q/k are stored as two tile sets:
  tile A: heads 0,1 at partition offsets 0,32 (the direct drain target)
  tile B: heads 2,3, DMA-shifted from A's partitions 64-127 down to 0-63
(PE matmuls with lhsT/rhs partition base 64/96 fail BIR/runtime; SBUF->
SBUF DMA moves across partitions instead). The qk psum chains emit the
channel order c = 64*(p//32) + 32*i + p%32 via host-side W column
permutation, so each drain stays partition-aligned. Drains are DVE
tensor_scalar (psum * QS/(XS*WS) + QS*bias -> fp8), with exp scale
1/QS^2 folded into the ACT activation.

With PE cut to ~65 us the Activation engine (exp: ~58 us of elements +
per-instr bubbles) becomes the critical engine; emission keeps ACT fed:
scores are emitted just-in-time ahead of their exps, and all bf16 PE
work (v chains, pv, transpose, proj) + qk chains ride as filler between
score slots. Masks run on GPSIMD(Pool), off the DVE/ACT critical paths.
"""

import os
import sys

for _p in ("/opt/trn_rl_repo", "/opt/pypackages"):
    if os.path.isdir(_p) and _p not in sys.path:
        sys.path.append(_p)

import numpy as np

import concourse.bass as bass
import concourse.tile as tile
import concourse.mybir as mybir
from concourse import bacc
from concourse.bass_utils import run_bass_kernel_spmd

B, T, C = 2, 2048, 1024
H = 16            # total heads
D = 64            # head dim
HPC = 4           # heads per core
CH = HPC * D      # 256 channels per core
N_CORES = 8

f32 = mybir.dt.float32
bf16 = mybir.dt.bfloat16
fp8 = mybir.dt.float8e4
ts = bass.ts
ds = bass.ds
AF = mybir.ActivationFunctionType
ALU = mybir.AluOpType
PM = mybir.MatmulPerfMode

XS = 8.0    # fp8 x pre-scale
WS = 64.0   # fp8 W pre-scale
QS = 2.0    # stored q/k fp8 scale
DRAIN_S = float(QS / (XS * WS))
EXP_S = float(1.0 / (QS * QS))

_COMPILED = None

POLICY = {
    "norm_lag": int(os.environ.get("K_NORM_LAG", "1")),
    "spend_proj": os.environ.get("K_SPEND_PROJ", "s3"),  # none|s3|s23|all
    "sc_bufs": int(os.environ.get("K_SC_BUFS", "2")),
    "big_bufs": int(os.environ.get("K_BIG_BUFS", "2")),
    "masks_pool": int(os.environ.get("K_MASKS_POOL", "1")),
    "v3_defer": int(os.environ.get("K_V3_DEFER", "1")),
    "out_bf16": int(os.environ.get("K_OUT_BF16", "1")),
    "adv_fill": int(os.environ.get("K_ADV_FILL", "1")),
    "py_bufs": int(os.environ.get("K_PY_BUFS", "2")),
    "xt_prefetch": int(os.environ.get("K_XT_PREFETCH", "1")),
    "drain_alt": int(os.environ.get("K_DRAIN_ALT", "2")),
    "tail_fine": int(os.environ.get("K_TAIL_FINE", "1")),
    "proj_merge": int(os.environ.get("K_PROJ_MERGE", "1")),
    "v_defer_all": int(os.environ.get("K_V_DEFER_ALL", "1")),
    "double_fill": int(os.environ.get("K_DOUBLE_FILL", "0")),
    "slot_swap": int(os.environ.get("K_SLOT_SWAP", "0")),
    "fp8_scores": int(os.environ.get("K_FP8_SCORES", "1")),  # fallback knob
    "warm_exp": int(os.environ.get("K_WARM_EXP", "1")),
    "bulk_q": os.environ.get("K_BULK_Q", "sp"),  # act|sp: bulk DMA queue
    "pv_split": int(os.environ.get("K_PV_SPLIT", "0")),
    "act_drain0": int(os.environ.get("K_ACT_DRAIN0", "0")),
}


def _build():
    nc = bacc.Bacc("TRN2", target_bir_lowering=False, debug=False,
                   num_devices=N_CORES)

    # DRAM inputs (host-prepped layouts)
    x8 = nc.dram_tensor("x8", [128, 4, 2, T], fp8, kind="ExternalInput").ap()
    wqk8 = nc.dram_tensor("wqk8", [128, 4, 2, 512], fp8,
                          kind="ExternalInput").ap()
    xT = nc.dram_tensor("xT", [C, T], bf16, kind="ExternalInput").ap()
    wv = nc.dram_tensor("wv", [128, 8, CH], bf16, kind="ExternalInput").ap()
    wpt = nc.dram_tensor("wpt", [CH, C], f32, kind="ExternalInput").ap()
    bqk = nc.dram_tensor("bqk", [128, 4], f32, kind="ExternalInput").ap()
    bvb = nc.dram_tensor("bvb", [128, HPC, D], f32, kind="ExternalInput").ap()
    Sm = nc.dram_tensor("Sm", [128, 128], f32, kind="ExternalInput").ap()
    Idm = nc.dram_tensor("Idm", [128, 128], f32, kind="ExternalInput").ap()
    out_dt = bf16 if POLICY["out_bf16"] else f32
    out = nc.dram_tensor("out_partial", [T, C], out_dt,
                         kind="ExternalOutput").ap()

    NT512 = T // 512          # 4   512-token stripes
    NT128 = T // 128          # 16  128-token tiles
    NC128 = C // 128          # 8   contraction tiles (bf16 v path)

    with tile.TileContext(nc) as tc:
        with tc.tile_pool(name="consts", bufs=1) as consts, \
             tc.tile_pool(name="qkv", bufs=1) as qkv, \
             tc.tile_pool(name="x8p",
                          bufs=POLICY["xt_prefetch"] + 1) as x8p, \
             tc.tile_pool(name="xp",
                          bufs=POLICY["xt_prefetch"] + 1) as xp, \
             tc.tile_pool(name="pp", bufs=17) as pp, \
             tc.tile_pool(name="yn", bufs=2) as yn, \
             tc.tile_pool(name="op", bufs=3) as op, \
             tc.tile_pool(name="ps_s", bufs=POLICY["sc_bufs"],
                          space="PSUM") as ps_s, \
             tc.tile_pool(name="ps_y", bufs=POLICY["py_bufs"],
                          space="PSUM") as ps_y, \
             tc.tile_pool(name="ps_big", bufs=POLICY["big_bufs"],
                          space="PSUM") as ps_big:

            bulk = nc.scalar if POLICY["bulk_q"] == "act" else nc.sync

            # ---- startup DMAs on the SP queue: bias first (tiny), then a
            #      small j=0 pair so the first DR chain matmul starts early,
            #      then the remaining chunks as two bigger transfers ----
            bqk_sb = consts.tile([128, 4], f32)
            nc.sync.dma_start(bqk_sb[:], bqk)
            wqk_sb = consts.tile([128, 4, 2, 512], fp8)
            x8t0 = x8p.tile([128, 4, 2, 512], fp8, tag="x8t")
            nc.sync.dma_start(wqk_sb[:], wqk8)
            nc.sync.dma_start(x8t0[:], x8[:, :, :, ts(0, 512)])

            # dummy exp: pulls LoadActFuncSet into the startup DMA window
            if POLICY["warm_exp"]:
                warm = consts.tile([128, 1], f32)
                nc.vector.memset(warm[:], 0.0)
                warm_o = consts.tile([128, 1], bf16)
                nc.scalar.activation(warm_o[:], warm[:], AF.Exp)

            xT_r = xT.rearrange("(o p) t -> p o t", p=128)
            xt0 = xp.tile([128, NC128, 512], bf16, tag="xt")
            wv_sb = consts.tile([128, 8, CH], bf16)
            bvb_sb = consts.tile([128, HPC, D], f32)
            S_f = consts.tile([128, 128], f32)
            Id_f = consts.tile([128, 128], f32)
            wpt_f = consts.tile([128, 2, C], f32)

            def emit_bulk_loads():
                # emitted after stripe-0 qk units so their shift DMAs get
                # transfer-priority over these bulk bytes
                bulk.dma_start(xt0[:], xT_r[:, :, ts(0, 512)])
                bulk.dma_start(wv_sb[:], wv)
                bulk.dma_start(bvb_sb[:], bvb)
                bulk.dma_start(S_f[:], Sm)
                bulk.dma_start(Id_f[:], Idm)
                bulk.dma_start(
                    wpt_f[:], wpt.rearrange("(s p) o -> p s o", p=128))
                nc.gpsimd.tensor_copy(S_sb[:], S_f[:])
                nc.gpsimd.tensor_copy(Id_sb[:], Id_f[:])
                nc.gpsimd.tensor_copy(wpt_sb[:], wpt_f[:])

            S_sb = consts.tile([128, 128], bf16)
            Id_sb = consts.tile([128, 128], bf16)
            wpt_sb = consts.tile([128, 2, C], bf16)

            # ---- persistent activations ----
            # q/k fp8: tile A holds heads 0,1 (parts 0..63) as drained;
            # B gets heads 2,3 DMA-shifted from A's parts 64..127.
            qT8a = qkv.tile([128, 2, T], fp8)
            qT8b = qkv.tile([128, 2, T], fp8)
            kT8a = qkv.tile([128, 2, T], fp8)
            kT8b = qkv.tile([128, 2, T], fp8)
            vaug = qkv.tile([128, NT128, HPC, D + 1], bf16)  # [kt,ki,h,d|1]
            yT = qkv.tile([128, 2, T], bf16)

            nc.vector.memset(vaug[:, :, :, D:D + 1], 1.0)

            # ---------------- emission helpers ----------------
            from collections import deque

            x8_tiles = {0: x8t0}
            xt_tiles = {0: xt0}

            def ensure_xt_dma(ti):
                if ti < NT512 and ti not in x8_tiles:
                    x8t = x8p.tile([128, 4, 2, 512], fp8, tag="x8t")
                    nc.sync.dma_start(x8t[:], x8[:, :, :, ts(ti, 512)])
                    x8_tiles[ti] = x8t
                if ti < NT512 and ti not in xt_tiles:
                    xt = xp.tile([128, NC128, 512], bf16, tag="xt")
                    nc.sync.dma_start(xt[:], xT_r[:, :, ts(ti, 512)])
                    xt_tiles[ti] = xt

            # chain cc: 0=q,i0  1=q,i1  2=k,i0  3=k,i1
            def emit_qk_chain(ti, cc):
                x8t = x8_tiles[ti]
                st = qT8a if cc < 2 else kT8a
                i = cc % 2
                ps = ps_big.tile([128, 512], f32, tag="big")
                for j in range(4):
                    nc.tensor.matmul(
                        ps[:], wqk_sb[:, j, :, ts(cc, 128)], x8t[:, j],
                        start=(j == 0), stop=(j == 3),
                        perf_mode=PM.DoubleRow)
                if ti == 0 and POLICY["act_drain0"] and cc in (0, 2):
                    # startup: ACT is idle; halve the serial drain latency
                    nc.scalar.activation(
                        st[:, i, ts(ti, 512)], ps[:], AF.Identity,
                        bias=bqk_sb[:, cc:cc + 1], scale=DRAIN_S)
                else:
                    nc.vector.tensor_scalar(
                        st[:, i, ts(ti, 512)], ps[:], DRAIN_S,
                        bqk_sb[:, cc:cc + 1], op0=ALU.mult, op1=ALU.add)

            def emit_qk_shift(ti, qk):
                # move heads 2,3 (parts 64..127) down to parts 0..63 of B
                a, b = (qT8a, qT8b) if qk == 0 else (kT8a, kT8b)
                nc.sync.dma_start(b[0:64, :, ts(ti, 512)],
                                  a[ds(64, 64), :, ts(ti, 512)])

            def qk_units(ti):
                return [(emit_qk_chain, (ti, 0)),
                        (emit_qk_chain, (ti, 1)),
                        (emit_qk_shift, (ti, 0)),
                        (emit_qk_chain, (ti, 2)),
                        (emit_qk_chain, (ti, 3)),
                        (emit_qk_shift, (ti, 1))]

            def emit_v_chain(ti, tj):
                xt = xt_tiles[ti]
                pv = ps_big.tile([128, HPC, D], f32, tag="big")
                for ci in range(NC128):
                    nc.tensor.matmul(
                        pv[:, :, :], xt[:, ci, ts(tj, 128)],
                        wv_sb[:, ci, :],
                        start=(ci == 0), stop=(ci == NC128 - 1))
                nc.vector.tensor_add(
                    out=vaug[:, 4 * ti + tj, :, 0:D],
                    in0=pv[:, :, :], in1=bvb_sb[:])

            p4_all = {}  # (stripe, ki) -> p4 tile

            def emit_scores(qi, ki, groups=(0, 1)):
                j = ki - 4 * qi
                q0 = max(0, 128 * j)
                w = 512 - q0
                if (qi, ki) in p4_all:
                    p4 = p4_all[(qi, ki)]
                else:
                    p4 = pp.tile([128, HPC, 512], bf16, tag="p4")
                for g in groups:
                    KT = kT8a if g == 0 else kT8b
                    QT = qT8a if g == 0 else qT8b
                    sc = ps_s.tile([128, 2, 512], f32, tag="sc")
                    for hh in range(2):
                        nc.tensor.matmul(
                            sc[:, hh, q0:],
                            KT[ts(hh, 32), :, ts(ki, 128)],
                            QT[ts(hh, 32), :, ds(512 * qi + q0, w)],
                            start=True, stop=True,
                            perf_mode=PM.DoubleRow,
                            tile_position=(32 * hh, 0))
                    nc.scalar.activation(
                        p4[:, ts(g, 2), q0:], sc[:, :, q0:], AF.Exp,
                        scale=EXP_S)
                    if j >= 0:
                        eng = nc.gpsimd if POLICY["masks_pool"] else nc.vector
                        for hh in range(2):
                            eng.tensor_mul(
                                out=p4[:, 2 * g + hh, q0:q0 + 128],
                                in0=p4[:, 2 * g + hh, q0:q0 + 128],
                                in1=S_sb[:])
                p4_all[(qi, ki)] = p4

            def emit_pv(tg, last_ki=None):
                # last_ki < tg leaves the accumulation groups open; a later
                # emit_pv_fin() adds the remaining k-blocks and closes them.
                tg_rel, qi = tg % 4, tg // 4
                if last_ki is None:
                    last_ki = tg
                py4 = ps_y.tile([128, HPC, D + 1], f32, tag="py")
                for h in range(HPC):
                    for ki in range(last_ki + 1):
                        nc.tensor.matmul(
                            py4[:, h, :],
                            p4_all[(qi, ki)][:, h, ts(tg_rel, 128)],
                            vaug[:, ki, h, :],
                            start=(ki == 0), stop=(ki == tg))
                py4s[tg] = py4
                pv_done[tg] = last_ki
                if not POLICY["norm_lag"] and last_ki == tg:
                    emit_norm(tg)

            def emit_pv_fin(tg):
                tg_rel, qi = tg % 4, tg // 4
                py4 = py4s[tg]
                for h in range(HPC):
                    for ki in range(pv_done[tg] + 1, tg + 1):
                        nc.tensor.matmul(
                            py4[:, h, :],
                            p4_all[(qi, ki)][:, h, ts(tg_rel, 128)],
                            vaug[:, ki, h, :],
                            start=(ki == 0), stop=(ki == tg))
                pv_done[tg] = tg

            pv_done = [None] * NT128

            def emit_norm(tg):
                py4 = py4s[tg]
                rec4 = yn.tile([128, HPC, 1], f32, tag="rec")
                nc.vector.reciprocal(rec4[:], py4[:, :, D:D + 1])
                y_n = yn.tile([128, HPC, D], bf16, tag="yn")
                nc.vector.tensor_mul(
                    out=y_n[:], in0=py4[:, :, 0:D],
                    in1=rec4.to_broadcast([128, HPC, D]))
                y_ns[tg] = y_n

            py4s = [None] * NT128
            y_ns = [None] * NT128

            def emit_transpose(tg):
                yTt = ps_y.tile([128, 2, 128], bf16, tag="py")
                for i in range(2):
                    nc.tensor.transpose(
                        yTt[:, i, :], y_ns[tg][:, ts(i, 2), :], Id_sb[:])
                nc.vector.tensor_copy(yT[:, :, ts(tg, 128)], yTt[:])
                proj_q.append(tg)

            def emit_proj(tg, split_drain=False):
                if POLICY["proj_merge"]:
                    pos2 = [ps_big.tile([128, 512], f32, tag="big",
                                        name=f"po_m{k}") for k in range(2)]
                    for oi in range(2):
                        for s in range(2):
                            nc.tensor.matmul(
                                pos2[oi][:], yT[:, s, ts(tg, 128)],
                                wpt_sb[:, s, ts(oi, 512)],
                                start=(s == 0), stop=(s == 1))
                    ot2 = op.tile([128, 2, 512], out_dt, tag="ot2")
                    nc.vector.tensor_copy(ot2[:, 0, :], pos2[0][:])
                    if split_drain and POLICY["drain_alt"] in (1, 2):
                        nc.scalar.activation(ot2[:, 1, :], pos2[1][:], AF.Copy)
                    else:
                        nc.vector.tensor_copy(ot2[:, 1, :], pos2[1][:])
                    nc.sync.dma_start(out[ts(tg, 128), :], ot2[:])
                    return
                for oi in range(2):
                    po = ps_big.tile([128, 512], f32, tag="big")
                    for s in range(2):
                        nc.tensor.matmul(
                            po[:], yT[:, s, ts(tg, 128)],
                            wpt_sb[:, s, ts(oi, 512)],
                            start=(s == 0), stop=(s == 1))
                    ot = op.tile([128, 512], out_dt, tag="ot")
                    nc.vector.tensor_copy(ot[:], po[:])
                    nc.sync.dma_start(out[ts(tg, 128), ts(oi, 512)], ot[:])

            # -------- software-pipelined emission --------
            filler = deque()   # pending PE-heavy units
            proj_q = deque()   # proj tiles ready to emit
            state = {"pv": 0}

            def advance_pipeline(upto, spend_proj=False):
                nl = POLICY["norm_lag"]
                sp = POLICY["pv_split"]
                while state["pv"] <= min(upto, NT128 - 1):
                    tg = state["pv"]
                    if sp:
                        if tg >= 1:
                            emit_pv_fin(tg - 1)
                            emit_norm(tg - 1)
                        if tg >= 2:
                            emit_transpose(tg - 2)
                        if spend_proj and proj_q:
                            emit_proj(proj_q.popleft())
                        emit_pv(tg, last_ki=tg - 1)
                    else:
                        if nl and tg >= 1:
                            emit_norm(tg - 1)
                        if tg >= 1 + nl:
                            emit_transpose(tg - 1 - nl)
                        if spend_proj and proj_q:
                            emit_proj(proj_q.popleft())
                        emit_pv(tg)
                    state["pv"] += 1

            for ti in range(NT512):
                if ti == 0:
                    for fn, args in qk_units(0):
                        fn(*args)
                    emit_bulk_loads()
                ensure_xt_dma(ti)
                for pf in range(1, POLICY["xt_prefetch"] + 1):
                    ensure_xt_dma(ti + pf)
                defer_v = (ti == 3 and POLICY["v3_defer"]) or \
                          POLICY["v_defer_all"]
                if defer_v:
                    for tj in range(3, -1, -1):
                        filler.appendleft((emit_v_chain, (ti, tj)))
                else:
                    for tj in range(4):
                        emit_v_chain(ti, tj)
                if ti + 1 < NT512:
                    for unit in qk_units(ti + 1):
                        filler.append(unit)
                if POLICY["adv_fill"] and ti > 0:
                    units = [(advance_pipeline, (m,))
                             for m in range(state["pv"], 4 * ti)]
                    for u in reversed(units):
                        filler.appendleft(u)
                else:
                    advance_pipeline(4 * ti - 1)

                nk = 4 * ti + 4
                spend = POLICY["spend_proj"]
                nfill = 1 + (POLICY["double_fill"] and ti == 3)
                for ki in range(nk):
                    def _slot_work():
                        if ki - 1 >= 4 * ti:
                            if ti == 0 and filler:
                                fn, args = filler.popleft()
                                fn(*args)
                            advance_pipeline(
                                ki - 1,
                                spend_proj=(spend == "all"
                                            or (spend in ("s3", "s23")
                                                and ti == 3)
                                            or (spend == "s23" and ti == 2)))
                        else:
                            for _ in range(nfill):
                                if filler:
                                    fn, args = filler.popleft()
                                    fn(*args)
                                elif proj_q:
                                    emit_proj(proj_q.popleft())
                    if POLICY["slot_swap"]:
                        _slot_work()
                        emit_scores(ti, ki)
                    else:
                        emit_scores(ti, ki)
                        _slot_work()
                while filler:
                    fn, args = filler.popleft()
                    fn(*args)

            def emit_tail_fine():
                # tile 15: per-slab norm -> transpose -> yT copy interleaved
                # with the proj contraction chain, to shorten the serial tail
                tg = NT128 - 1
                py4 = py4s[tg]
                y_n = yn.tile([128, HPC, D], bf16, tag="yn")
                yTt = ps_y.tile([128, 2, 128], bf16, tag="py")
                pos = [ps_big.tile([128, 512], f32, tag="big", name=f"po_t{k}")
                       for k in range(2)]
                for s in range(2):
                    rec2 = yn.tile([128, 2, 1], f32, tag="rec")
                    nc.vector.reciprocal(
                        rec2[:], py4[:, ts(s, 2), D:D + 1])
                    nc.vector.tensor_mul(
                        out=y_n[:, ts(s, 2), :], in0=py4[:, ts(s, 2), 0:D],
                        in1=rec2.to_broadcast([128, 2, D]))
                    nc.tensor.transpose(
                        yTt[:, s, :], y_n[:, ts(s, 2), :], Id_sb[:])
                    nc.vector.tensor_copy(
                        yT[:, s, ts(tg, 128)], yTt[:, s, :])
                    for oi in range(2):
                        nc.tensor.matmul(
                            pos[oi][:], yT[:, s, ts(tg, 128)],
                            wpt_sb[:, s, ts(oi, 512)],
                            start=(s == 0), stop=(s == 1))
                ot2 = op.tile([128, 2, 512], out_dt, tag="ot2")
                nc.vector.tensor_copy(ot2[:, 0, :], pos[0][:])
                if POLICY["drain_alt"] in (1, 2):
                    nc.scalar.activation(ot2[:, 1, :], pos[1][:], AF.Copy)
                else:
                    nc.vector.tensor_copy(ot2[:, 1, :], pos[1][:])
                nc.sync.dma_start(out[ts(tg, 128), :], ot2[:])

            advance_pipeline(NT128 - 1)
            if POLICY["pv_split"]:
                # state: pv(15) open at ki<=14, norm done <=14, transp <=13
                emit_transpose(NT128 - 2)
                while proj_q:
                    emit_proj(proj_q.popleft(),
                              split_drain=POLICY["drain_alt"] == 2)
                emit_pv_fin(NT128 - 1)
                if POLICY["tail_fine"]:
                    emit_tail_fine()
                else:
                    emit_norm(NT128 - 1)
                    emit_transpose(NT128 - 1)
                    emit_proj(NT128 - 1,
                              split_drain=POLICY["drain_alt"] == 2)
            elif POLICY["norm_lag"]:
                if POLICY["tail_fine"]:
                    emit_transpose(NT128 - 2)
                    while proj_q:
                        emit_proj(proj_q.popleft(),
                                  split_drain=POLICY["drain_alt"] == 2)
                    emit_tail_fine()
                else:
                    emit_norm(NT128 - 1)
                    emit_transpose(NT128 - 2)
            if not POLICY["pv_split"] and \
                    not (POLICY["norm_lag"] and POLICY["tail_fine"]):
                emit_transpose(NT128 - 1)
                while proj_q:
                    emit_proj(proj_q.popleft(),
                              split_drain=POLICY["drain_alt"] == 2)

    nc.compile()
    return nc


def _get_compiled():
    global _COMPILED
    if _COMPILED is None:
        _COMPILED = _build()
    return _COMPILED


def _host_prep(x, W_attn, b_attn, W_proj, b_proj):
    import ml_dtypes
    scale = 1.0 / np.sqrt(np.float32(D))
    xTb = [np.ascontiguousarray(x[b].T).astype(np.float32) for b in range(B)]
    Sm = (np.arange(128, dtype=np.int32)[None, :]
          >= np.arange(128, dtype=np.int32)[:, None]).astype(np.float32)
    Idm = np.eye(128, dtype=np.float32)

    # channel order for a qk chain with sub-row i: psum partition p' holds
    # local channel c = 64*(p'//32) + 32*i + (p'%32)
    pp_ = np.arange(128)
    c_of_p = {i: 64 * (pp_ // 32) + 32 * i + (pp_ % 32) for i in (0, 1)}

    in_maps = []
    for c in range(N_CORES):
        b, g = divmod(c, 4)
        ch = slice(CH * g, CH * (g + 1))
        Wq = W_attn[ch]                    # [256, C]
        Wk = W_attn[C:][ch] * scale
        Wv = W_attn[2 * C:][ch]
        bq = b_attn[ch]
        bk = b_attn[C:][ch] * scale
        bv = b_attn[2 * C:][ch]

        # x8: [128, 4(j), 2(i), T] = xT[(2j+i)*128+p, t] * XS
        x8_c = np.ascontiguousarray(
            (xTb[b].reshape(4, 2, 128, T).transpose(2, 0, 1, 3) * XS)
        ).astype(ml_dtypes.float8_e4m3)

        # wqk8: [128(p), 4(j), 2(i_row), 512(4 chains x 128 cols)]
        # chain cc: 0=q,i0 1=q,i1 2=k,i0 3=k,i1; col p' -> channel c_of_p
        wqk = np.empty((C, 4, 128), dtype=np.float32)  # [row, chain, col]
        wqk[:, 0, :] = Wq[c_of_p[0]].T
        wqk[:, 1, :] = Wq[c_of_p[1]].T
        wqk[:, 2, :] = Wk[c_of_p[0]].T
        wqk[:, 3, :] = Wk[c_of_p[1]].T
        wqk = wqk.reshape(4, 2, 128, 4 * 128).transpose(2, 0, 1, 3)
        wqk8_c = np.ascontiguousarray(wqk * WS).astype(ml_dtypes.float8_e4m3)

        # bqk: [128, 4] = QS * bias[channel(p', chain)]
        bqk_c = np.empty((128, 4), dtype=np.float32)
        bqk_c[:, 0] = QS * bq[c_of_p[0]]
        bqk_c[:, 1] = QS * bq[c_of_p[1]]
        bqk_c[:, 2] = QS * bk[c_of_p[0]]
        bqk_c[:, 3] = QS * bk[c_of_p[1]]

        wv_c = np.ascontiguousarray(
            Wv.T.reshape(8, 128, CH).transpose(1, 0, 2)
        ).astype(ml_dtypes.bfloat16)
        bvb_c = np.ascontiguousarray(
            np.broadcast_to(bv[None, :].reshape(1, HPC, D),
                            (128, HPC, D))).astype(np.float32)
        wpt_c = np.ascontiguousarray(W_proj[:, ch].T).astype(np.float32)

        im = {
            "x8": x8_c,
            "wqk8": wqk8_c,
            "xT": xTb[b].astype(ml_dtypes.bfloat16),
            "wv": wv_c,
            "wpt": wpt_c,
            "bqk": bqk_c,
            "bvb": bvb_c,
            "Sm": Sm,
            "Idm": Idm,
        }
        in_maps.append(im)
    return in_maps


def kernel(x, W_attn, b_attn, W_proj, b_proj):
    x = np.asarray(x, dtype=np.float32)
    W_attn = np.asarray(W_attn, dtype=np.float32)
    b_attn = np.asarray(b_attn, dtype=np.float32)
    W_proj = np.asarray(W_proj, dtype=np.float32)
    b_proj = np.asarray(b_proj, dtype=np.float32)

    nc = _get_compiled()
    in_maps = _host_prep(x, W_attn, b_attn, W_proj, b_proj)
    res = run_bass_kernel_spmd(nc, in_maps, core_ids=list(range(N_CORES)))

    out = np.empty((B, T, C), dtype=np.float32)
    for b in range(B):
        acc = np.asarray(res.results[4 * b]["out_partial"],
                         dtype=np.float32).copy()
        for g in range(1, 4):
            acc += np.asarray(res.results[4 * b + g]["out_partial"],
                              dtype=np.float32)
        out[b] = acc + b_proj
    return out


# revision 17
# speedup vs baseline: 1.0986x; 1.0276x over previous
"""Causal self-attention on 8 NeuronCores (Bass/Tile).

Sharding: tensor-parallel over heads x data-parallel over batch.
  core c -> batch b = c//4, heads 4g..4g+3 where g = c%4.
Each core computes q,k,v for its 4 heads (over its batch's 2048 tokens),
causal softmax attention, and the partial output projection over its 256
head-channels. Host sums the 4 partials per batch and adds b_proj.

v3 design: fp8(e4m3) DoubleRow matmuls for the q/k projection chains and
the score matmuls (cost model: DoubleRow fp8 = 0.5 cyc/row with 2x128
contraction per instruction -> 4x cheaper qk projection, 2x cheaper
scores). Numerics (measured vs f32 reference): ~1.65e-2 max-rel, under
the 2e-2 gate. v/pv/proj stay bf16 (fp8 there fails the gate).

Layout for fp8 scores: per head the contraction is d=64, split as
[32 partitions x 2 DoubleRow sub-rows]. q/k are stored as two tile sets:
  tile A: heads 0,1 at partition offsets 0,32 (the direct drain target)
  tile B: heads 2,3, DMA-shifted from A's partitions 64-127 down to 0-63
(PE matmuls with lhsT/rhs partition base 64/96 fail BIR/runtime; SBUF->
SBUF DMA moves across partitions instead). The qk psum chains emit the
channel order c = 64*(p//32) + 32*i + p%32 via host-side W column
permutation, so each drain stays partition-aligned. Drains are DVE
tensor_scalar (psum * QS/(XS*WS) + QS*bias -> fp8), with exp scale
1/QS^2 folded into the ACT activation.

With PE cut to ~65 us the Activation engine (exp: ~58 us of elements +
per-instr bubbles) becomes the critical engine; emission keeps ACT fed:
scores are emitted just-in-time ahead of their exps, and all bf16 PE
work (v chains, pv, transpose, proj) + qk chains ride as filler between
score slots. Masks run on GPSIMD(Pool), off the DVE/ACT critical paths.
"""

import os
import sys

for _p in ("/opt/trn_rl_repo", "/opt/pypackages"):
    if os.path.isdir(_p) and _p not in sys.path:
        sys.path.append(_p)

import numpy as np

import concourse.bass as bass
import concourse.tile as tile
import concourse.mybir as mybir
from concourse import bacc
from concourse.bass_utils import run_bass_kernel_spmd

B, T, C = 2, 2048, 1024
H = 16            # total heads
D = 64            # head dim
HPC = 4           # heads per core
CH = HPC * D      # 256 channels per core
N_CORES = 8

f32 = mybir.dt.float32
bf16 = mybir.dt.bfloat16
fp8 = mybir.dt.float8e4
ts = bass.ts
ds = bass.ds
AF = mybir.ActivationFunctionType
ALU = mybir.AluOpType
PM = mybir.MatmulPerfMode

XS = 8.0    # fp8 x pre-scale
WS = 64.0   # fp8 W pre-scale
QS = 2.0    # stored q/k fp8 scale
DRAIN_S = float(QS / (XS * WS))
EXP_S = float(1.0 / (QS * QS))

_COMPILED = None

POLICY = {
    "norm_lag": int(os.environ.get("K_NORM_LAG", "1")),
    "spend_proj": os.environ.get("K_SPEND_PROJ", "s3"),  # none|s3|s23|all
    "sc_bufs": int(os.environ.get("K_SC_BUFS", "2")),
    "big_bufs": int(os.environ.get("K_BIG_BUFS", "2")),
    "masks_pool": int(os.environ.get("K_MASKS_POOL", "1")),
    "v3_defer": int(os.environ.get("K_V3_DEFER", "1")),
    "out_bf16": int(os.environ.get("K_OUT_BF16", "1")),
    "adv_fill": int(os.environ.get("K_ADV_FILL", "1")),
    "py_bufs": int(os.environ.get("K_PY_BUFS", "2")),
    "xt_prefetch": int(os.environ.get("K_XT_PREFETCH", "1")),
    "drain_alt": int(os.environ.get("K_DRAIN_ALT", "2")),
    "tail_fine": int(os.environ.get("K_TAIL_FINE", "1")),
    "proj_merge": int(os.environ.get("K_PROJ_MERGE", "1")),
    "v_defer_all": int(os.environ.get("K_V_DEFER_ALL", "1")),
    "double_fill": int(os.environ.get("K_DOUBLE_FILL", "0")),
    "slot_swap": int(os.environ.get("K_SLOT_SWAP", "0")),
    "fp8_scores": int(os.environ.get("K_FP8_SCORES", "1")),  # fallback knob
    "warm_exp": int(os.environ.get("K_WARM_EXP", "1")),
    "bulk_q": os.environ.get("K_BULK_Q", "sp"),  # act|sp: bulk DMA queue
    "pv_split": int(os.environ.get("K_PV_SPLIT", "0")),
    "act_drain0": int(os.environ.get("K_ACT_DRAIN0", "1")),
    "warmup": int(os.environ.get("K_WARMUP", "19")),
}


def _build():
    nc = bacc.Bacc("TRN2", target_bir_lowering=False, debug=False,
                   num_devices=N_CORES)

    # DRAM inputs (host-prepped layouts)
    x8 = nc.dram_tensor("x8", [128, 4, 2, T], fp8, kind="ExternalInput").ap()
    wqk8 = nc.dram_tensor("wqk8", [128, 4, 2, 512], fp8,
                          kind="ExternalInput").ap()
    xT = nc.dram_tensor("xT", [C, T], bf16, kind="ExternalInput").ap()
    wv = nc.dram_tensor("wv", [128, 8, CH], bf16, kind="ExternalInput").ap()
    wpt = nc.dram_tensor("wpt", [CH, C], f32, kind="ExternalInput").ap()
    bqk = nc.dram_tensor("bqk", [128, 4], f32, kind="ExternalInput").ap()
    bvb = nc.dram_tensor("bvb", [128, HPC, D], f32, kind="ExternalInput").ap()
    Sm = nc.dram_tensor("Sm", [128, 128], f32, kind="ExternalInput").ap()
    Idm = nc.dram_tensor("Idm", [128, 128], f32, kind="ExternalInput").ap()
    out_dt = bf16 if POLICY["out_bf16"] else f32
    out = nc.dram_tensor("out_partial", [T, C], out_dt,
                         kind="ExternalOutput").ap()

    NT512 = T // 512          # 4   512-token stripes
    NT128 = T // 128          # 16  128-token tiles
    NC128 = C // 128          # 8   contraction tiles (bf16 v path)

    with tile.TileContext(nc) as tc:
        with tc.tile_pool(name="consts", bufs=1) as consts, \
             tc.tile_pool(name="qkv", bufs=1) as qkv, \
             tc.tile_pool(name="x8p",
                          bufs=POLICY["xt_prefetch"] + 1) as x8p, \
             tc.tile_pool(name="xp",
                          bufs=POLICY["xt_prefetch"] + 1) as xp, \
             tc.tile_pool(name="pp", bufs=17) as pp, \
             tc.tile_pool(name="yn", bufs=2) as yn, \
             tc.tile_pool(name="op", bufs=3) as op, \
             tc.tile_pool(name="ps_s", bufs=POLICY["sc_bufs"],
                          space="PSUM") as ps_s, \
             tc.tile_pool(name="ps_y", bufs=POLICY["py_bufs"],
                          space="PSUM") as ps_y, \
             tc.tile_pool(name="ps_big", bufs=POLICY["big_bufs"],
                          space="PSUM") as ps_big:

            bulk = nc.scalar if POLICY["bulk_q"] == "act" else nc.sync

            # ---- startup DMAs on the SP queue: bias first (tiny), then a
            #      small j=0 pair so the first DR chain matmul starts early,
            #      then the remaining chunks as two bigger transfers ----
            bqk_sb = consts.tile([128, 4], f32)
            nc.sync.dma_start(bqk_sb[:], bqk)
            wqk_sb = consts.tile([128, 4, 2, 512], fp8)
            x8t0 = x8p.tile([128, 4, 2, 512], fp8, tag="x8t")
            nc.sync.dma_start(wqk_sb[:], wqk8)
            nc.sync.dma_start(x8t0[:], x8[:, :, :, ts(0, 512)])

            # dummy exp: pulls LoadActFuncSet into the startup DMA window
            if POLICY["warm_exp"]:
                warm = consts.tile([128, 1], f32)
                nc.vector.memset(warm[:], 0.0)
                warm_o = consts.tile([128, 1], bf16)
                nc.scalar.activation(warm_o[:], warm[:], AF.Exp)

            if POLICY["warmup"]:
                # dummy matmuls: keep PE continuously busy through the
                # startup DMA wait so the p-state ramp completes before the
                # first real chains (and they run at full clock).
                wz = consts.tile([128, 512], bf16)
                nc.gpsimd.memset(wz[:], 0.0)
                for _ in range(POLICY["warmup"]):
                    wps = ps_big.tile([128, 512], f32, tag="big")
                    nc.tensor.matmul(wps[:], wz[:, :128], wz[:],
                                     start=True, stop=True)

            xT_r = xT.rearrange("(o p) t -> p o t", p=128)
            xt0 = xp.tile([128, NC128, 512], bf16, tag="xt")
            wv_sb = consts.tile([128, 8, CH], bf16)
            bvb_sb = consts.tile([128, HPC, D], f32)
            S_f = consts.tile([128, 128], f32)
            Id_f = consts.tile([128, 128], f32)
            wpt_f = consts.tile([128, 2, C], f32)

            def emit_bulk_loads():
                # emitted after stripe-0 qk units so their shift DMAs get
                # transfer-priority; ordered by first-use time:
                # S (stripe-0 diag masks ~10us), x8(1) handled by caller,
                # xt0/wv (v(0) fillers ~12us), Id (transposes ~18us),
                # wpt (first proj ~30us)
                ensure_xt_dma(1, x8_only=True)
                bulk.dma_start(S_f[:], Sm)
                nc.gpsimd.tensor_copy(S_sb[:], S_f[:])
                bulk.dma_start(xt0[:], xT_r[:, :, ts(0, 512)])
                bulk.dma_start(wv_sb[:], wv)
                bulk.dma_start(bvb_sb[:], bvb)
                bulk.dma_start(Id_f[:], Idm)
                nc.gpsimd.tensor_copy(Id_sb[:], Id_f[:])
                bulk.dma_start(
                    wpt_f[:], wpt.rearrange("(s p) o -> p s o", p=128))
                nc.gpsimd.tensor_copy(wpt_sb[:], wpt_f[:])

            S_sb = consts.tile([128, 128], bf16)
            Id_sb = consts.tile([128, 128], bf16)
            wpt_sb = consts.tile([128, 2, C], bf16)

            # ---- persistent activations ----
            # q/k fp8: tile A holds heads 0,1 (parts 0..63) as drained;
            # B gets heads 2,3 DMA-shifted from A's parts 64..127.
            qT8a = qkv.tile([128, 2, T], fp8)
            qT8b = qkv.tile([128, 2, T], fp8)
            kT8a = qkv.tile([128, 2, T], fp8)
            kT8b = qkv.tile([128, 2, T], fp8)
            vaug = qkv.tile([128, NT128, HPC, D + 1], bf16)  # [kt,ki,h,d|1]
            yT = qkv.tile([128, 2, T], bf16)

            nc.vector.memset(vaug[:, :, :, D:D + 1], 1.0)

            # ---------------- emission helpers ----------------
            from collections import deque

            x8_tiles = {0: x8t0}
            xt_tiles = {0: xt0}

            def ensure_xt_dma(ti, x8_only=False):
                if ti < NT512 and ti not in x8_tiles:
                    x8t = x8p.tile([128, 4, 2, 512], fp8, tag="x8t")
                    nc.sync.dma_start(x8t[:], x8[:, :, :, ts(ti, 512)])
                    x8_tiles[ti] = x8t
                if x8_only:
                    return
                if ti < NT512 and ti not in xt_tiles:
                    xt = xp.tile([128, NC128, 512], bf16, tag="xt")
                    nc.sync.dma_start(xt[:], xT_r[:, :, ts(ti, 512)])
                    xt_tiles[ti] = xt

            # chain cc: 0=q,i0  1=q,i1  2=k,i0  3=k,i1
            def emit_qk_chain(ti, cc):
                x8t = x8_tiles[ti]
                st = qT8a if cc < 2 else kT8a
                i = cc % 2
                ps = ps_big.tile([128, 512], f32, tag="big")
                for j in range(4):
                    nc.tensor.matmul(
                        ps[:], wqk_sb[:, j, :, ts(cc, 128)], x8t[:, j],
                        start=(j == 0), stop=(j == 3),
                        perf_mode=PM.DoubleRow)
                if ti == 0 and POLICY["act_drain0"] and cc in (0, 2):
                    # startup: ACT is idle; halve the serial drain latency
                    nc.scalar.activation(
                        st[:, i, ts(ti, 512)], ps[:], AF.Identity,
                        bias=bqk_sb[:, cc:cc + 1], scale=DRAIN_S)
                else:
                    nc.vector.tensor_scalar(
                        st[:, i, ts(ti, 512)], ps[:], DRAIN_S,
                        bqk_sb[:, cc:cc + 1], op0=ALU.mult, op1=ALU.add)

            def emit_qk_shift(ti, qk):
                # move heads 2,3 (parts 64..127) down to parts 0..63 of B
                a, b = (qT8a, qT8b) if qk == 0 else (kT8a, kT8b)
                nc.sync.dma_start(b[0:64, :, ts(ti, 512)],
                                  a[ds(64, 64), :, ts(ti, 512)])

            def qk_units(ti):
                return [(emit_qk_chain, (ti, 0)),
                        (emit_qk_chain, (ti, 1)),
                        (emit_qk_shift, (ti, 0)),
                        (emit_qk_chain, (ti, 2)),
                        (emit_qk_chain, (ti, 3)),
                        (emit_qk_shift, (ti, 1))]

            def emit_v_chain(ti, tj):
                xt = xt_tiles[ti]
                pv = ps_big.tile([128, HPC, D], f32, tag="big")
                for ci in range(NC128):
                    nc.tensor.matmul(
                        pv[:, :, :], xt[:, ci, ts(tj, 128)],
                        wv_sb[:, ci, :],
                        start=(ci == 0), stop=(ci == NC128 - 1))
                nc.vector.tensor_add(
                    out=vaug[:, 4 * ti + tj, :, 0:D],
                    in0=pv[:, :, :], in1=bvb_sb[:])

            p4_all = {}  # (stripe, ki) -> p4 tile

            def emit_scores(qi, ki, groups=(0, 1)):
                j = ki - 4 * qi
                q0 = max(0, 128 * j)
                w = 512 - q0
                if (qi, ki) in p4_all:
                    p4 = p4_all[(qi, ki)]
                else:
                    p4 = pp.tile([128, HPC, 512], bf16, tag="p4")
                for g in groups:
                    KT = kT8a if g == 0 else kT8b
                    QT = qT8a if g == 0 else qT8b
                    sc = ps_s.tile([128, 2, 512], f32, tag="sc")
                    for hh in range(2):
                        nc.tensor.matmul(
                            sc[:, hh, q0:],
                            KT[ts(hh, 32), :, ts(ki, 128)],
                            QT[ts(hh, 32), :, ds(512 * qi + q0, w)],
                            start=True, stop=True,
                            perf_mode=PM.DoubleRow,
                            tile_position=(32 * hh, 0))
                    nc.scalar.activation(
                        p4[:, ts(g, 2), q0:], sc[:, :, q0:], AF.Exp,
                        scale=EXP_S)
                    if j >= 0:
                        eng = nc.gpsimd if POLICY["masks_pool"] else nc.vector
                        for hh in range(2):
                            eng.tensor_mul(
                                out=p4[:, 2 * g + hh, q0:q0 + 128],
                                in0=p4[:, 2 * g + hh, q0:q0 + 128],
                                in1=S_sb[:])
                p4_all[(qi, ki)] = p4

            def emit_pv(tg, last_ki=None):
                # last_ki < tg leaves the accumulation groups open; a later
                # emit_pv_fin() adds the remaining k-blocks and closes them.
                tg_rel, qi = tg % 4, tg // 4
                if last_ki is None:
                    last_ki = tg
                py4 = ps_y.tile([128, HPC, D + 1], f32, tag="py")
                for h in range(HPC):
                    for ki in range(last_ki + 1):
                        nc.tensor.matmul(
                            py4[:, h, :],
                            p4_all[(qi, ki)][:, h, ts(tg_rel, 128)],
                            vaug[:, ki, h, :],
                            start=(ki == 0), stop=(ki == tg))
                py4s[tg] = py4
                pv_done[tg] = last_ki
                if not POLICY["norm_lag"] and last_ki == tg:
                    emit_norm(tg)

            def emit_pv_fin(tg):
                tg_rel, qi = tg % 4, tg // 4
                py4 = py4s[tg]
                for h in range(HPC):
                    for ki in range(pv_done[tg] + 1, tg + 1):
                        nc.tensor.matmul(
                            py4[:, h, :],
                            p4_all[(qi, ki)][:, h, ts(tg_rel, 128)],
                            vaug[:, ki, h, :],
                            start=(ki == 0), stop=(ki == tg))
                pv_done[tg] = tg

            pv_done = [None] * NT128

            def emit_norm(tg):
                py4 = py4s[tg]
                rec4 = yn.tile([128, HPC, 1], f32, tag="rec")
                nc.vector.reciprocal(rec4[:], py4[:, :, D:D + 1])
                y_n = yn.tile([128, HPC, D], bf16, tag="yn")
                nc.vector.tensor_mul(
                    out=y_n[:], in0=py4[:, :, 0:D],
                    in1=rec4.to_broadcast([128, HPC, D]))
                y_ns[tg] = y_n

            py4s = [None] * NT128
            y_ns = [None] * NT128

            def emit_transpose(tg):
                yTt = ps_y.tile([128, 2, 128], bf16, tag="py")
                for i in range(2):
                    nc.tensor.transpose(
                        yTt[:, i, :], y_ns[tg][:, ts(i, 2), :], Id_sb[:])
                nc.vector.tensor_copy(yT[:, :, ts(tg, 128)], yTt[:])
                proj_q.append(tg)

            def emit_proj(tg, split_drain=False):
                if POLICY["proj_merge"]:
                    pos2 = [ps_big.tile([128, 512], f32, tag="big",
                                        name=f"po_m{k}") for k in range(2)]
                    for oi in range(2):
                        for s in range(2):
                            nc.tensor.matmul(
                                pos2[oi][:], yT[:, s, ts(tg, 128)],
                                wpt_sb[:, s, ts(oi, 512)],
                                start=(s == 0), stop=(s == 1))
                    ot2 = op.tile([128, 2, 512], out_dt, tag="ot2")
                    nc.vector.tensor_copy(ot2[:, 0, :], pos2[0][:])
                    if split_drain and POLICY["drain_alt"] in (1, 2):
                        nc.scalar.activation(ot2[:, 1, :], pos2[1][:], AF.Copy)
                    else:
                        nc.vector.tensor_copy(ot2[:, 1, :], pos2[1][:])
                    nc.sync.dma_start(out[ts(tg, 128), :], ot2[:])
                    return
                for oi in range(2):
                    po = ps_big.tile([128, 512], f32, tag="big")
                    for s in range(2):
                        nc.tensor.matmul(
                            po[:], yT[:, s, ts(tg, 128)],
                            wpt_sb[:, s, ts(oi, 512)],
                            start=(s == 0), stop=(s == 1))
                    ot = op.tile([128, 512], out_dt, tag="ot")
                    nc.vector.tensor_copy(ot[:], po[:])
                    nc.sync.dma_start(out[ts(tg, 128), ts(oi, 512)], ot[:])

            # -------- software-pipelined emission --------
            filler = deque()   # pending PE-heavy units
            proj_q = deque()   # proj tiles ready to emit
            state = {"pv": 0}

            def advance_pipeline(upto, spend_proj=False):
                nl = POLICY["norm_lag"]
                sp = POLICY["pv_split"]
                while state["pv"] <= min(upto, NT128 - 1):
                    tg = state["pv"]
                    if sp:
                        if tg >= 1:
                            emit_pv_fin(tg - 1)
                            emit_norm(tg - 1)
                        if tg >= 2:
                            emit_transpose(tg - 2)
                        if spend_proj and proj_q:
                            emit_proj(proj_q.popleft())
                        emit_pv(tg, last_ki=tg - 1)
                    else:
                        if nl and tg >= 1:
                            emit_norm(tg - 1)
                        if tg >= 1 + nl:
                            emit_transpose(tg - 1 - nl)
                        if spend_proj and proj_q:
                            emit_proj(proj_q.popleft())
                        emit_pv(tg)
                    state["pv"] += 1

            for ti in range(NT512):
                if ti == 0:
                    for fn, args in qk_units(0):
                        fn(*args)
                    emit_bulk_loads()
                ensure_xt_dma(ti)
                for pf in range(1, POLICY["xt_prefetch"] + 1):
                    ensure_xt_dma(ti + pf)
                defer_v = (ti == 3 and POLICY["v3_defer"]) or \
                          POLICY["v_defer_all"]
                if defer_v:
                    for tj in range(3, -1, -1):
                        filler.appendleft((emit_v_chain, (ti, tj)))
                else:
                    for tj in range(4):
                        emit_v_chain(ti, tj)
                if ti + 1 < NT512:
                    for unit in qk_units(ti + 1):
                        filler.append(unit)
                if POLICY["adv_fill"] and ti > 0:
                    units = [(advance_pipeline, (m,))
                             for m in range(state["pv"], 4 * ti)]
                    for u in reversed(units):
                        filler.appendleft(u)
                else:
                    advance_pipeline(4 * ti - 1)

                nk = 4 * ti + 4
                spend = POLICY["spend_proj"]
                nfill = 1 + (POLICY["double_fill"] and ti == 3)
                for ki in range(nk):
                    def _slot_work():
                        if ki - 1 >= 4 * ti:
                            if ti == 0 and filler:
                                fn, args = filler.popleft()
                                fn(*args)
                            advance_pipeline(
                                ki - 1,
                                spend_proj=(spend == "all"
                                            or (spend in ("s3", "s23")
                                                and ti == 3)
                                            or (spend == "s23" and ti == 2)))
                        else:
                            for _ in range(nfill):
                                if filler:
                                    fn, args = filler.popleft()
                                    fn(*args)
                                elif proj_q:
                                    emit_proj(proj_q.popleft())
                    if POLICY["slot_swap"]:
                        _slot_work()
                        emit_scores(ti, ki)
                    else:
                        emit_scores(ti, ki)
                        _slot_work()
                while filler:
                    fn, args = filler.popleft()
                    fn(*args)

            def emit_tail_fine():
                # tile 15: per-slab norm -> transpose -> yT copy interleaved
                # with the proj contraction chain, to shorten the serial tail
                tg = NT128 - 1
                py4 = py4s[tg]
                y_n = yn.tile([128, HPC, D], bf16, tag="yn")
                yTt = ps_y.tile([128, 2, 128], bf16, tag="py")
                pos = [ps_big.tile([128, 512], f32, tag="big", name=f"po_t{k}")
                       for k in range(2)]
                for s in range(2):
                    rec2 = yn.tile([128, 2, 1], f32, tag="rec")
                    nc.vector.reciprocal(
                        rec2[:], py4[:, ts(s, 2), D:D + 1])
                    nc.vector.tensor_mul(
                        out=y_n[:, ts(s, 2), :], in0=py4[:, ts(s, 2), 0:D],
                        in1=rec2.to_broadcast([128, 2, D]))
                    nc.tensor.transpose(
                        yTt[:, s, :], y_n[:, ts(s, 2), :], Id_sb[:])
                    nc.vector.tensor_copy(
                        yT[:, s, ts(tg, 128)], yTt[:, s, :])
                    for oi in range(2):
                        nc.tensor.matmul(
                            pos[oi][:], yT[:, s, ts(tg, 128)],
                            wpt_sb[:, s, ts(oi, 512)],
                            start=(s == 0), stop=(s == 1))
                ot2 = op.tile([128, 2, 512], out_dt, tag="ot2")
                nc.vector.tensor_copy(ot2[:, 0, :], pos[0][:])
                if POLICY["drain_alt"] in (1, 2):
                    nc.scalar.activation(ot2[:, 1, :], pos[1][:], AF.Copy)
                else:
                    nc.vector.tensor_copy(ot2[:, 1, :], pos[1][:])
                nc.sync.dma_start(out[ts(tg, 128), :], ot2[:])

            advance_pipeline(NT128 - 1)
            if POLICY["pv_split"]:
                # state: pv(15) open at ki<=14, norm done <=14, transp <=13
                emit_transpose(NT128 - 2)
                while proj_q:
                    emit_proj(proj_q.popleft(),
                              split_drain=POLICY["drain_alt"] == 2)
                emit_pv_fin(NT128 - 1)
                if POLICY["tail_fine"]:
                    emit_tail_fine()
                else:
                    emit_norm(NT128 - 1)
                    emit_transpose(NT128 - 1)
                    emit_proj(NT128 - 1,
                              split_drain=POLICY["drain_alt"] == 2)
            elif POLICY["norm_lag"]:
                if POLICY["tail_fine"]:
                    emit_transpose(NT128 - 2)
                    while proj_q:
                        emit_proj(proj_q.popleft(),
                                  split_drain=POLICY["drain_alt"] == 2)
                    emit_tail_fine()
                else:
                    emit_norm(NT128 - 1)
                    emit_transpose(NT128 - 2)
            if not POLICY["pv_split"] and \
                    not (POLICY["norm_lag"] and POLICY["tail_fine"]):
                emit_transpose(NT128 - 1)
                while proj_q:
                    emit_proj(proj_q.popleft(),
                              split_drain=POLICY["drain_alt"] == 2)

    nc.compile()
    return nc


def _get_compiled():
    global _COMPILED
    if _COMPILED is None:
        _COMPILED = _build()
    return _COMPILED


def _host_prep(x, W_attn, b_attn, W_proj, b_proj):
    import ml_dtypes
    scale = 1.0 / np.sqrt(np.float32(D))
    xTb = [np.ascontiguousarray(x[b].T).astype(np.float32) for b in range(B)]
    Sm = (np.arange(128, dtype=np.int32)[None, :]
          >= np.arange(128, dtype=np.int32)[:, None]).astype(np.float32)
    Idm = np.eye(128, dtype=np.float32)

    # channel order for a qk chain with sub-row i: psum partition p' holds
    # local channel c = 64*(p'//32) + 32*i + (p'%32)
    pp_ = np.arange(128)
    c_of_p = {i: 64 * (pp_ // 32) + 32 * i + (pp_ % 32) for i in (0, 1)}

    in_maps = []
    for c in range(N_CORES):
        b, g = divmod(c, 4)
        ch = slice(CH * g, CH * (g + 1))
        Wq = W_attn[ch]                    # [256, C]
        Wk = W_attn[C:][ch] * scale
        Wv = W_attn[2 * C:][ch]
        bq = b_attn[ch]
        bk = b_attn[C:][ch] * scale
        bv = b_attn[2 * C:][ch]

        # x8: [128, 4(j), 2(i), T] = xT[(2j+i)*128+p, t] * XS
        x8_c = np.ascontiguousarray(
            (xTb[b].reshape(4, 2, 128, T).transpose(2, 0, 1, 3) * XS)
        ).astype(ml_dtypes.float8_e4m3)

        # wqk8: [128(p), 4(j), 2(i_row), 512(4 chains x 128 cols)]
        # chain cc: 0=q,i0 1=q,i1 2=k,i0 3=k,i1; col p' -> channel c_of_p
        wqk = np.empty((C, 4, 128), dtype=np.float32)  # [row, chain, col]
        wqk[:, 0, :] = Wq[c_of_p[0]].T
        wqk[:, 1, :] = Wq[c_of_p[1]].T
        wqk[:, 2, :] = Wk[c_of_p[0]].T
        wqk[:, 3, :] = Wk[c_of_p[1]].T
        wqk = wqk.reshape(4, 2, 128, 4 * 128).transpose(2, 0, 1, 3)
        wqk8_c = np.ascontiguousarray(wqk * WS).astype(ml_dtypes.float8_e4m3)

        # bqk: [128, 4] = QS * bias[channel(p', chain)]
        bqk_c = np.empty((128, 4), dtype=np.float32)
        bqk_c[:, 0] = QS * bq[c_of_p[0]]
        bqk_c[:, 1] = QS * bq[c_of_p[1]]
        bqk_c[:, 2] = QS * bk[c_of_p[0]]
        bqk_c[:, 3] = QS * bk[c_of_p[1]]

        wv_c = np.ascontiguousarray(
            Wv.T.reshape(8, 128, CH).transpose(1, 0, 2)
        ).astype(ml_dtypes.bfloat16)
        bvb_c = np.ascontiguousarray(
            np.broadcast_to(bv[None, :].reshape(1, HPC, D),
                            (128, HPC, D))).astype(np.float32)
        wpt_c = np.ascontiguousarray(W_proj[:, ch].T).astype(np.float32)

        im = {
            "x8": x8_c,
            "wqk8": wqk8_c,
            "xT": xTb[b].astype(ml_dtypes.bfloat16),
            "wv": wv_c,
            "wpt": wpt_c,
            "bqk": bqk_c,
            "bvb": bvb_c,
            "Sm": Sm,
            "Idm": Idm,
        }
        in_maps.append(im)
    return in_maps


def kernel(x, W_attn, b_attn, W_proj, b_proj):
    x = np.asarray(x, dtype=np.float32)
    W_attn = np.asarray(W_attn, dtype=np.float32)
    b_attn = np.asarray(b_attn, dtype=np.float32)
    W_proj = np.asarray(W_proj, dtype=np.float32)
    b_proj = np.asarray(b_proj, dtype=np.float32)

    nc = _get_compiled()
    in_maps = _host_prep(x, W_attn, b_attn, W_proj, b_proj)
    res = run_bass_kernel_spmd(nc, in_maps, core_ids=list(range(N_CORES)))

    out = np.empty((B, T, C), dtype=np.float32)
    for b in range(B):
        acc = np.asarray(res.results[4 * b]["out_partial"],
                         dtype=np.float32).copy()
        for g in range(1, 4):
            acc += np.asarray(res.results[4 * b + g]["out_partial"],
                              dtype=np.float32)
        out[b] = acc + b_proj
    return out
